# revision 11
# baseline (speedup 1.0000x reference)
"""Trainium2 Bass kernel v2 for nn_BiModel (2-layer bidirectional GCN).

Distribution over 8 NeuronCores, nodes sharded 6250/core.

Structure (vs v1):
- Each layer AllGathers its bf16 message table in two per-core half
  slices so the second collective overlaps with aggregation of the first
  half; gathers read the collective output directly (no DRAM copy).
- Node halves are block-aligned and padded: half 0 = local rows [0,3200),
  half 1 = [3200,6250) padded to 3072 rows.  Half tables hold
  8*3200=25600 / 8*3072=24576 rows (int16-gatherable).
- Edge chunks are capacity-padded per (branch, src-half, dst-block); both
  branches' one-hot matmuls pair into a single PSUM tile per dst block.
- Layer 2 aggregates prescaled h2 directly; W_last applied afterwards.
"""

import numpy as np

import concourse.bass as bass
import concourse.bacc as bacc
import concourse.mybir as mybir
import concourse.tile as tile
from concourse.bass_utils import run_bass_kernel_spmd
from concourse.masks import make_identity

import ml_dtypes

P = 128
F32 = mybir.dt.float32
BF16 = mybir.dt.bfloat16
I16 = mybir.dt.int16
I32 = mybir.dt.int32

FULL_CFG = dict(n=50000, e=800000, f_in=500, h=64, c_out=16, n_cores=8,
                cap_floor=5, blocks_per_group=5, hl=3200)


def cdiv(a, b):
    return (a + b - 1) // b


# ----------------------------------------------------------------------------
# host-side layout / preprocessing
# ----------------------------------------------------------------------------

class Layout2:
    """Compile-time layout shared by all cores (uniform SPMD program).
    Edge chunk capacity per (branch, src-half, dst-block) = max count over
    cores rounded up to 128 chunks, floored at cap_floor chunks."""

    def __init__(self, cfg, counts):
        # counts: [n_cores, 2, 2, nblk] (core, branch, src-half, dst-blk)
        self.cfg = cfg
        self.nloc = cfg["n"] // cfg["n_cores"]
        self.nblk = cdiv(self.nloc, P)
        hl = cfg["hl"]
        self.hpad = [hl, self.nblk * P - hl]
        self.htot = [hp * cfg["n_cores"] for hp in self.hpad]
        cap = counts.max(axis=0)                      # [2, 2, nblk]
        self.cap_chunks = np.maximum(cdiv(cap, P), cfg["cap_floor"])
        self.chunk_off = np.zeros((2, 2, self.nblk), np.int64)
        self.nchunks_bh = np.zeros((2, 2), np.int64)
        for b in range(2):
            for h in range(2):
                off = 0
                for blk in range(self.nblk):
                    self.chunk_off[b, h, blk] = off
                    off += self.cap_chunks[b, h, blk]
                self.nchunks_bh[b, h] = off
        bg = cfg["blocks_per_group"]
        self.groups = [list(range(g * bg, min((g + 1) * bg, self.nblk)))
                       for g in range(cdiv(self.nblk, bg))]

    def signature(self):
        return (tuple(self.cap_chunks.reshape(-1).tolist()),
                tuple(sorted(self.cfg.items())))


def _wrap_idx16(idx, n_pad):
    buf = np.zeros(n_pad, np.int16)
    buf[: len(idx)] = idx.astype(np.int16)
    w = buf.reshape(n_pad // 16, 16).T            # [16, n/16]
    return np.ascontiguousarray(np.tile(w, (8, 1)))  # [128, n/16]


def host_prep(cfg, x, edge_index, is_reversed):
    n, f_in = cfg["n"], cfg["f_in"]
    n_cores = cfg["n_cores"]
    nloc = n // n_cores
    nblk = cdiv(nloc, P)
    f_pad = cdiv(f_in, P) * P
    kch = f_pad // P
    hl = cfg["hl"]
    hpad = [hl, nblk * P - hl]

    src = np.asarray(edge_index[0], np.int64)
    dst = np.asarray(edge_index[1], np.int64)
    rev = np.asarray(is_reversed).astype(bool)

    core = dst // nloc
    dl = dst % nloc
    blk = dl // P
    branch = rev.astype(np.int64)
    cs = src // nloc
    rs = src % nloc
    hf = (rs >= hl).astype(np.int64)               # src half
    tblidx = cs * np.where(hf == 0, hpad[0], hpad[1]) + (rs - hf * hl)

    key = (((core * 2 + branch) * 2 + hf) * nblk) + blk
    order = np.argsort(key, kind="stable")
    counts = np.bincount(key[order], minlength=n_cores * 2 * 2 * nblk)
    counts = counts.reshape(n_cores, 2, 2, nblk)
    lay = Layout2(cfg, counts)

    deg = np.zeros((2, n), np.float32)
    np.add.at(deg[0], dst[~rev], 1.0)
    np.add.at(deg[1], dst[rev], 1.0)

    # node-feature transpose, bf16, padded
    xT = np.zeros((f_pad, n), ml_dtypes.bfloat16)
    xT[:f_in] = np.asarray(x, np.float32).T

    tbl_s = tblidx[order]
    dl_s = dl[order]
    gs = np.concatenate([[0], np.cumsum(counts.reshape(-1))])[:-1]
    gs = gs.reshape(n_cores, 2, 2, nblk)

    nblk_pad = nblk * P
    in_maps = []
    for c in range(n_cores):
        xc = xT[:, c * nloc:(c + 1) * nloc].reshape(kch, P, nloc)
        m = {"xT": np.ascontiguousarray(
            xc.transpose(1, 0, 2).reshape(P, kch * nloc))}
        degs = np.ones((P, 2 * nblk), np.float32)
        for b in range(2):
            dloc = np.ones(nblk_pad, np.float32)
            dloc[:nloc] = deg[b, c * nloc:(c + 1) * nloc]
            degs[:, b * nblk:(b + 1) * nblk] = dloc.reshape(nblk, P).T
        m["degs"] = degs
        for b in range(2):
            for h in range(2):
                nch = max(int(lay.nchunks_bh[b, h]), 1)
                tot = nch * P
                idx_stream = np.zeros(tot, np.int16)
                dstv = np.full(tot, -1.0, np.float32)
                for blk_ in range(nblk):
                    cnt = int(counts[c, b, h, blk_])
                    s0 = int(gs[c, b, h, blk_])
                    co = int(lay.chunk_off[b, h, blk_]) * P
                    idx_stream[co:co + cnt] = tbl_s[s0:s0 + cnt]
                    dstv[co:co + cnt] = dl_s[s0:s0 + cnt] - blk_ * P
                m[f"idx_b{b}h{h}"] = _wrap_idx16(idx_stream, tot)
                m[f"dst_b{b}h{h}"] = np.ascontiguousarray(
                    dstv.reshape(nch, P).T)          # [128, nch]
        in_maps.append(m)
    return lay, in_maps


def host_prep_weights(cfg, W_st0, b_st0, W_ts0, b_ts0, W_st1, b_st1,
                      W_ts1, b_ts1, W_last, b_last):
    f_in, h, c_out = cfg["f_in"], cfg["h"], cfg["c_out"]
    f_pad = cdiv(f_in, P) * P
    W0 = np.zeros((f_pad, 2 * h), np.float32)
    W0[:f_in, :h] = W_st0
    W0[:f_in, h:] = W_ts0
    kch = f_pad // P
    W0 = np.ascontiguousarray(
        W0.reshape(kch, P, 2 * h).transpose(1, 0, 2).reshape(P, kch * 2 * h))
    W1 = np.concatenate([W_st1, W_ts1], axis=1).astype(np.float32)
    WL = np.zeros((2 * h, 128), np.float32)
    WL[:, :c_out] = W_last
    bias01 = np.stack([np.concatenate([b_st0, b_ts0]),
                       np.concatenate([b_st1, b_ts1])], axis=1).astype(np.float32)
    return dict(W0=W0.astype(ml_dtypes.bfloat16),
                W1=W1.astype(ml_dtypes.bfloat16),
                WL=WL.astype(ml_dtypes.bfloat16), WLf=WL, bias01=bias01,
                b_last=np.asarray(b_last, np.float32).reshape(c_out, 1))


# ----------------------------------------------------------------------------
# device program
# ----------------------------------------------------------------------------

def build_program(cfg, lay, repeat=1, layers=3, no_coll=False):
    n, f_in = cfg["n"], cfg["f_in"]
    h, c_out = cfg["h"], cfg["c_out"]
    n_cores = cfg["n_cores"]
    nloc = n // n_cores
    nblk = lay.nblk
    nblk_pad = nblk * P
    f_pad = cdiv(f_in, P) * P
    kch = f_pad // P
    h2 = 2 * h
    hpad = lay.hpad
    htot = lay.htot
    hblk = [hpad[0] // P, hpad[1] // P]           # local blocks per half
    core_ids = list(range(n_cores))

    nc = bacc.Bacc("TRN2", target_bir_lowering=False, debug=False,
                   num_devices=n_cores)

    xT_d = nc.declare_dram_parameter("xT", [P, kch * nloc], BF16, isOutput=False)
    degs_d = nc.declare_dram_parameter("degs", [P, 2 * nblk], F32, isOutput=False)
    W0_d = nc.declare_dram_parameter("W0", [P, kch * h2], BF16, isOutput=False)
    W1_d = nc.declare_dram_parameter("W1", [h2, h2], BF16, isOutput=False)
    WL_d = nc.declare_dram_parameter("WL", [h2, 128], BF16, isOutput=False)
    WLf_d = nc.declare_dram_parameter("WLf", [h2, 128], F32, isOutput=False)
    bias01_d = nc.declare_dram_parameter("bias01", [h2, 2], F32, isOutput=False)
    b_last_d = nc.declare_dram_parameter("b_last", [c_out, 1], F32, isOutput=False)
    idx_d, dst_d = {}, {}
    for b in range(2):
        for hf in range(2):
            ncw = max(int(lay.nchunks_bh[b, hf]), 1)
            idx_d[b, hf] = nc.declare_dram_parameter(
                f"idx_b{b}h{hf}", [P, ncw * 8], I16, isOutput=False)
            dst_d[b, hf] = nc.declare_dram_parameter(
                f"dst_b{b}h{hf}", [P, ncw], F32, isOutput=False)
    out_d = nc.declare_dram_parameter("out", [nloc, c_out], F32, isOutput=True)

    tbl_loc_h = [nc.dram_tensor(f"tblloc_h{i}", [hpad[i], h2], BF16)
                 for i in range(2)]
    tbl_half = [nc.dram_tensor(f"tbl_h{i}", [htot[i], h2], BF16,
                               addr_space="Shared") for i in range(2)]

    with tile.TileContext(nc) as tc:
        with (
            tc.tile_pool(name="persist", bufs=1) as pp,
            tc.tile_pool(name="init", bufs=1) as ip,
            tc.tile_pool(name="work", bufs=2) as wp,
            tc.tile_pool(name="xload", bufs=3) as xp,
            tc.tile_pool(name="msg", bufs=2) as mp,
            tc.tile_pool(name="mask", bufs=6) as kp,
            tc.tile_pool(name="psA", bufs=3, space="PSUM") as psA,
            tc.tile_pool(name="psN", bufs=2, space="PSUM") as psN,
            tc.tile_pool(name="psB", bufs=2, space="PSUM") as psB,
        ):
            # ---------- constants ----------
            iota_i = ip.tile([P, P], I32, tag="ioi")
            nc.gpsimd.iota(iota_i[:], pattern=[[1, P]], base=0,
                           channel_multiplier=0)
            iota_bf = pp.tile([P, P], BF16, tag="iobf")
            nc.vector.tensor_copy(iota_bf[:], iota_i[:])
            ident = pp.tile([P, P], F32, tag="ident")
            make_identity(nc, ident[:])
            ident_bf = pp.tile([P, P], BF16, tag="identbf")
            nc.vector.tensor_copy(ident_bf[:], ident[:])
            biasv = pp.tile([P, 2], F32, tag="biasv")
            nc.sync.dma_start(out=biasv[:], in_=bias01_d[:, :])
            biasL = pp.tile([c_out, 1], F32, tag="biasL")
            nc.sync.dma_start(out=biasL[:], in_=b_last_d[:, :])

            # ---------- local degrees -> dinv [128, 3*nblk] (st|ts|all) ----
            deg_sb = ip.tile([P, 2 * nblk], F32, tag="degsb")
            nc.sync.dma_start(out=deg_sb[:], in_=degs_d[:, :])
            dtmp = ip.tile([P, 3 * nblk], F32, tag="dtmp")
            nc.vector.tensor_tensor(out=dtmp[:, 2 * nblk:],
                                    in0=deg_sb[:, :nblk], in1=deg_sb[:, nblk:],
                                    op=mybir.AluOpType.add)
            nc.vector.tensor_copy(dtmp[:, :2 * nblk], deg_sb[:])
            nc.vector.tensor_scalar_add(dtmp[:], dtmp[:], 1.0)
            dsq = ip.tile([P, 3 * nblk], F32, tag="dsq")
            nc.scalar.sqrt(dsq[:], dtmp[:])
            dinv = pp.tile([P, 3 * nblk], F32, tag="dinv")
            nc.vector.reciprocal(dinv[:], dsq[:])


            # dinv broadcast rows [128, nblk_pad] (st rows 0:64, ts 64:128)
            # and [c_out, nblk_pad] of dinv_all
            dinvT = ip.tile([nblk, 3 * P], F32, tag="dinvT")
            for i in range(3):
                tps = psB.tile([nblk, P], F32, tag="pst")
                nc.tensor.transpose(tps[:], dinv[:, i * nblk:(i + 1) * nblk],
                                    ident[:])
                nc.scalar.copy(dinvT[:, i * P:(i + 1) * P], tps[:])
            dinv_flat_d = nc.dram_tensor("dinv_flat", [1, 3 * nblk_pad], F32)
            for i in range(3):
                nc.sync.dma_start(
                    out=dinv_flat_d[0:1, i * nblk_pad:(i + 1) * nblk_pad],
                    in_=dinvT[:, i * P:(i + 1) * P])
            ones_row = pp.tile([1, P], F32, tag="ones_row")
            nc.vector.memset(ones_row[:], 1.0)
            dinvb = pp.tile([P, nblk_pad], F32, tag="dinvb")
            dinvallb = pp.tile([c_out, nblk_pad], F32, tag="dinvallb")
            NTB = 512
            for t0 in range(0, nblk_pad, NTB):
                t1 = min(t0 + NTB, nblk_pad)
                dfs = ip.tile([1, 3 * NTB], F32, tag="dfs")
                for i in range(3):
                    nc.sync.dma_start(
                        out=dfs[0:1, i * NTB: i * NTB + t1 - t0],
                        in_=dinv_flat_d[0:1, i * nblk_pad + t0: i * nblk_pad + t1])
                bps = psB.tile([P, NTB], F32, tag="pst")
                nc.tensor.matmul(bps[0:h, :t1 - t0], lhsT=ones_row[0:1, 0:h],
                                 rhs=dfs[0:1, 0:t1 - t0],
                                 start=True, stop=True)
                nc.tensor.matmul(bps[h:h2, :t1 - t0], lhsT=ones_row[0:1, 0:h],
                                 rhs=dfs[0:1, NTB:NTB + t1 - t0],
                                 start=True, stop=True, tile_position=(0, h))
                nc.scalar.copy(dinvb[:, t0:t1], bps[:, :t1 - t0])
                bps2 = psB.tile([P, NTB], F32, tag="pst")
                nc.tensor.matmul(bps2[:c_out, :t1 - t0],
                                 lhsT=ones_row[0:1, 0:c_out],
                                 rhs=dfs[0:1, 2 * NTB:2 * NTB + t1 - t0],
                                 start=True, stop=True)
                nc.scalar.copy(dinvallb[:, t0:t1], bps2[:c_out, :t1 - t0])

            # ---------- weights ----------
            w0_sb = pp.tile([P, kch * h2], BF16, tag="w0")
            nc.sync.dma_start(out=w0_sb[:], in_=W0_d[:, :])
            w1_sb = pp.tile([P, h2], BF16, tag="w1")
            nc.sync.dma_start(out=w1_sb[:], in_=W1_d[:, :])
            wl_sb = pp.tile([P, 128], BF16, tag="wl")
            nc.sync.dma_start(out=wl_sb[:], in_=WL_d[:, :])
            wlf_sb = pp.tile([P, 128], F32, tag="wlf")
            nc.sync.dma_start(out=wlf_sb[:], in_=WLf_d[:, :])

            # ---------- state ----------
            hT = pp.tile([P, nblk_pad], BF16, tag="hT")
            h2T = pp.tile([P, nblk_pad], BF16, tag="h2T")
            xwT = pp.tile([P, nblk_pad], F32, tag="xwT")
            aggT = pp.tile([P, nblk_pad], F32, tag="aggT")
            xwTL = xwT[0:c_out, :]
            outTL = aggT[0:c_out, :]

            # ---------------------------------------------------------------
            def build_xw_featmajor(src_getter, src_kch, w_ap_of_k, rows, dst):
                NT = 512
                for t0 in range(0, nloc, NT):
                    t1 = min(t0 + NT, nloc)
                    ps = psB.tile([P, NT], F32, tag="pst")
                    for k in range(src_kch):
                        nc.tensor.matmul(
                            ps[:rows, :t1 - t0],
                            lhsT=w_ap_of_k(k)[:, :rows],
                            rhs=src_getter(k, t0, t1),
                            start=(k == 0), stop=(k == src_kch - 1))
                    nc.scalar.copy(dst[:rows, t0:t1], ps[:rows, :t1 - t0])

            def build_table_rows(src_ap, w_ap, tbl_dst, blk_lo, blk_hi,
                                 row_base, prescale):
                """node-major prescaled bf16 table rows from feature-major
                SBUF source (single k chunk); all blocks full (padded)."""
                for blk in range(blk_lo, blk_hi):
                    nb0 = blk * P
                    ps = psN.tile([P, h2], F32, tag="psnm")
                    nc.tensor.matmul(ps[:, :], lhsT=src_ap[:, nb0:nb0 + P],
                                     rhs=w_ap, start=True, stop=True)
                    tt = wp.tile([P, h2], BF16, tag="tblt")
                    for (c0, c1, dcol) in prescale:
                        nc.vector.tensor_scalar_mul(
                            tt[:, c0:c1], ps[:, c0:c1],
                            dinv[:, dcol * nblk + blk: dcol * nblk + blk + 1])
                    nc.sync.dma_start(
                        out=tbl_dst[nb0 - row_base:nb0 - row_base + P, 0:h2],
                        in_=tt[:, :])

            # ---------------------------------------------------------------
            def aggregate(tables, full_rows):
                """Gather + one-hot-matmul segment sums over the 4 edge
                streams (branch x src-half).  full_rows=False: branch b uses
                lhs cols/psum rows [b*h,(b+1)*h).  full_rows=True (layer 2):
                full 128-wide lhs, both branches accumulate into all rows."""
                nmask = [0]

                def build_mask(dstcol_ap):
                    mk = kp.tile([P, P], BF16, tag="mask")
                    eng = nc.vector if nmask[0] % 3 else nc.gpsimd
                    nmask[0] += 1
                    eng.tensor_scalar(
                        out=mk[:], in0=iota_bf[:], scalar1=dstcol_ap,
                        scalar2=None, op0=mybir.AluOpType.is_equal)
                    return mk

                for hf in range(2):
                    for blocks in lay.groups:
                        bufs, dls = {}, {}
                        for b in range(2):
                            ch0 = int(lay.chunk_off[b, hf, blocks[0]])
                            ch1 = int(lay.chunk_off[b, hf, blocks[-1]]
                                      + lay.cap_chunks[b, hf, blocks[-1]])
                            nch = ch1 - ch0
                            it = wp.tile([P, nch * 8], I16, tag=f"idx{b}")
                            nc.sync.dma_start(
                                out=it[:], in_=idx_d[b, hf][:, ch0 * 8: ch1 * 8])
                            dt = wp.tile([P, nch], F32, tag=f"dl{b}")
                            nc.sync.dma_start(
                                out=dt[:], in_=dst_d[b, hf][:, ch0:ch1])
                            buf = mp.tile([P, nch, h2], BF16, tag=f"msg{b}")
                            nidx = nch * P
                            nc.gpsimd.dma_gather(
                                out_ap=buf[:], in_ap=tables[hf][:, :],
                                idxs_ap=it[:], num_idxs=nidx,
                                num_idxs_reg=nidx, elem_size=h2,
                                single_packet=(nidx <= 1024))
                            bufs[b] = (buf, ch0)
                            dls[b] = (dt, ch0)
                        for blk in blocks:
                            nb = slice(blk * P, (blk + 1) * P)
                            ps = psA.tile([P, P], F32, tag="agg")
                            for b in range(2):
                                buf, ch0 = bufs[b]
                                dt, _ = dls[b]
                                ncap = int(lay.cap_chunks[b, hf, blk])
                                co = int(lay.chunk_off[b, hf, blk])
                                for j in range(ncap):
                                    mk = build_mask(
                                        dt[:, co - ch0 + j: co - ch0 + j + 1])
                                    if full_rows:
                                        lh = buf[:, co - ch0 + j, 0:h2]
                                        o = ps[:, :]
                                        tpos = None
                                        st = (b == 0 and j == 0)
                                        sp = (b == 1 and j == ncap - 1)
                                    else:
                                        lh = buf[:, co - ch0 + j,
                                                 b * h:(b + 1) * h]
                                        o = ps[b * h:(b + 1) * h, :]
                                        tpos = (0, b * h)
                                        st = (j == 0)
                                        sp = (j == ncap - 1)
                                    nc.tensor.matmul(o, lhsT=lh, rhs=mk[:],
                                                     start=st, stop=sp,
                                                     tile_position=tpos)
                            if hf == 0:
                                nc.scalar.copy(aggT[:, nb], ps[:, :])
                            else:
                                nc.vector.tensor_add(out=aggT[:, nb],
                                                     in0=aggT[:, nb],
                                                     in1=ps[:, :])

            # ---------------------------------------------------------------
            def post01(layer, out_tile):
                nc.vector.tensor_tensor(out=xwT[:, :], in0=xwT[:, :],
                                        in1=dinvb[:, :],
                                        op=mybir.AluOpType.mult)
                nc.vector.tensor_tensor(out=aggT[:, :], in0=aggT[:, :],
                                        in1=xwT[:, :],
                                        op=mybir.AluOpType.add)
                nc.vector.tensor_tensor(out=aggT[:, :], in0=aggT[:, :],
                                        in1=dinvb[:, :],
                                        op=mybir.AluOpType.mult)
                nc.scalar.activation(out_tile[:, :], aggT[:, :],
                                     mybir.ActivationFunctionType.Relu,
                                     bias=biasv[:, layer:layer + 1])

            # ---------------------------------------------------------------
            def _phases():
                if nblk_pad > nloc:
                    nc.vector.memset(xwT[:, nloc:], 0.0)
                    nc.vector.memset(hT[:, nloc:], 0.0)
                    nc.vector.memset(h2T[:, nloc:], 0.0)

                # =================== layer 0 ===================
                with nc.named_scope("L0_tables"):
                    # zero padded tail rows of the half-1 local table once
                    zpad = hpad[1] * P - (nloc - hpad[0] * 1)
                    padrows = hpad[0] + hpad[1] - nloc      # 22
                    if padrows > 0:
                        zt = wp.tile([padrows, h2], BF16, tag="zpad")
                        nc.vector.memset(zt[:], 0.0)
                        nc.sync.dma_start(
                            out=tbl_loc_h[1][hpad[1] - padrows:hpad[1], :],
                            in_=zt[:])
                    NT = 512
                    for t0 in range(0, nloc, NT):
                        t1 = min(t0 + NT, nloc)
                        xts = []
                        for k in range(kch):
                            t = xp.tile([P, NT], BF16, tag=f"xb{k}")
                            nc.sync.dma_start(
                                out=t[:, :t1 - t0],
                                in_=xT_d[:, k * nloc + t0: k * nloc + t1])
                            xts.append(t)
                        # feature-major xw for the self-loop term
                        ps = psB.tile([P, NT], F32, tag="pst")
                        for k in range(kch):
                            nc.tensor.matmul(
                                ps[:h2, :t1 - t0],
                                lhsT=w0_sb[:, k * h2:(k + 1) * h2],
                                rhs=xts[k][:, :t1 - t0],
                                start=(k == 0), stop=(k == kch - 1))
                        nc.scalar.copy(xwT[:h2, t0:t1], ps[:h2, :t1 - t0])
                        # node-major prescaled table rows
                        for blk in range(t0 // P, cdiv(t1, P)):
                            nb0 = blk * P
                            nn = min(P, nloc - nb0)
                            psn = psN.tile([P, h2], F32, tag="psnm")
                            for k in range(kch):
                                nc.tensor.matmul(
                                    psn[:nn, :],
                                    lhsT=xts[k][:, nb0 - t0:nb0 - t0 + nn],
                                    rhs=w0_sb[:, k * h2:(k + 1) * h2],
                                    start=(k == 0), stop=(k == kch - 1))
                            tt = wp.tile([P, h2], BF16, tag="tblt")
                            for (cc0, cc1, dcol) in ((0, h, 0), (h, h2, 1)):
                                nc.vector.tensor_scalar_mul(
                                    tt[:nn, cc0:cc1], psn[:nn, cc0:cc1],
                                    dinv[:nn, dcol * nblk + blk:
                                         dcol * nblk + blk + 1])
                            hf = 0 if blk < hblk[0] else 1
                            r0 = nb0 - (hpad[0] if hf else 0)
                            nc.sync.dma_start(
                                out=tbl_loc_h[hf][r0:r0 + nn, :],
                                in_=tt[:nn, :])
                    if not no_coll:
                        for hf in range(2):
                            nc.gpsimd.collective_compute(
                                "AllGather", mybir.AluOpType.bypass,
                                replica_groups=[core_ids],
                                ins=[tbl_loc_h[hf][:]], outs=[tbl_half[hf][:]])
                with nc.named_scope("L0_agg"):
                    aggregate(tbl_half, False)
                with nc.named_scope("L0_post"):
                    post01(0, hT)
                if layers <= 1:
                    nc.sync.dma_start(out=out_d[0:P, :], in_=xwT[0:P, 0:c_out])
                    return

                # =================== layer 1 ===================
                with nc.named_scope("L1_tables"):
                    for hf in range(2):
                        build_table_rows(hT, w1_sb[:], tbl_loc_h[hf],
                                         0 if hf == 0 else hblk[0],
                                         hblk[0] if hf == 0 else nblk,
                                         0 if hf == 0 else hpad[0],
                                         ((0, h, 0), (h, h2, 1)))
                        if not no_coll:
                            nc.gpsimd.collective_compute(
                                "AllGather", mybir.AluOpType.bypass,
                                replica_groups=[core_ids],
                                ins=[tbl_loc_h[hf][:]], outs=[tbl_half[hf][:]])
                    build_xw_featmajor(lambda k, a, bb: hT[:, a:bb], 1,
                                       lambda k: w1_sb[:], h2, xwT)
                with nc.named_scope("L1_agg"):
                    aggregate(tbl_half, False)
                with nc.named_scope("L1_post"):
                    post01(1, h2T)
                if layers <= 2:
                    nc.sync.dma_start(out=out_d[0:P, :], in_=xwT[0:P, 0:c_out])
                    return

                # =================== layer 2 ===================
                with nc.named_scope("L2_tables"):
                    for hf in range(2):
                        build_table_rows(h2T, ident_bf[:], tbl_loc_h[hf],
                                         0 if hf == 0 else hblk[0],
                                         hblk[0] if hf == 0 else nblk,
                                         0 if hf == 0 else hpad[0],
                                         ((0, h2, 2),))
                        if not no_coll:
                            nc.gpsimd.collective_compute(
                                "AllGather", mybir.AluOpType.bypass,
                                replica_groups=[core_ids],
                                ins=[tbl_loc_h[hf][:]], outs=[tbl_half[hf][:]])
                    build_xw_featmajor(lambda k, a, bb: h2T[:, a:bb], 1,
                                       lambda k: wl_sb[:], c_out, xwT)
                with nc.named_scope("L2_agg"):
                    aggregate(tbl_half, True)

                # out16 = (WL^T aggT)*dinvall + xwTL*dinvall^2 + b_last
                with nc.named_scope("L2_post"):
                    NT = 512
                    for t0 in range(0, nblk_pad, NT):
                        t1 = min(t0 + NT, nblk_pad)
                        ps = psB.tile([P, NT], F32, tag="pst")
                        nc.tensor.matmul(ps[:c_out, :t1 - t0],
                                         lhsT=wlf_sb[:, :c_out],
                                         rhs=aggT[:, t0:t1],
                                         start=True, stop=True)
                        nc.scalar.copy(outTL[:, t0:t1], ps[:c_out, :t1 - t0])
                    nc.vector.tensor_tensor(out=xwTL[:, :], in0=xwTL[:, :],
                                            in1=dinvallb[:, :],
                                            op=mybir.AluOpType.mult)
                    nc.vector.tensor_tensor(out=outTL[:, :], in0=outTL[:, :],
                                            in1=xwTL[:, :],
                                            op=mybir.AluOpType.add)
                    nc.vector.tensor_tensor(out=outTL[:, :], in0=outTL[:, :],
                                            in1=dinvallb[:, :],
                                            op=mybir.AluOpType.mult)
                    nc.scalar.activation(outTL[:, :], outTL[:, :],
                                         mybir.ActivationFunctionType.Identity,
                                         bias=biasL[:, 0:1])

                with nc.named_scope("softmax"):
                    for blk in range(nblk):
                        nb0 = blk * P
                        nb1 = min(nb0 + P, nloc)
                        nn = nb1 - nb0
                        if nn <= 0:
                            continue
                        tp = psB.tile([P, c_out], F32, tag="pst")
                        nc.tensor.transpose(tp[:], outTL[:, nb0:nb0 + P],
                                            ident[:c_out, :c_out])
                        negmax = wp.tile([P, 1], F32, tag="negmax")
                        nc.vector.tensor_reduce(negmax[:], tp[:],
                                                axis=mybir.AxisListType.X,
                                                op=mybir.AluOpType.max,
                                                negate=True)
                        ex = wp.tile([P, c_out], F32, tag="ex")
                        nc.scalar.activation(ex[:], tp[:],
                                             mybir.ActivationFunctionType.Exp,
                                             bias=negmax[:, 0:1])
                        sume = wp.tile([P, 1], F32, tag="sume")
                        nc.vector.tensor_reduce(sume[:], ex[:],
                                                axis=mybir.AxisListType.X,
                                                op=mybir.AluOpType.add)
                        lse = wp.tile([P, 1], F32, tag="lse")
                        nc.scalar.activation(lse[:], sume[:],
                                             mybir.ActivationFunctionType.Ln)
                        fin = wp.tile([P, c_out], F32, tag="fin")
                        nc.vector.tensor_scalar(
                            out=fin[:], in0=tp[:], scalar1=negmax[:, 0:1],
                            scalar2=lse[:, 0:1], op0=mybir.AluOpType.add,
                            op1=mybir.AluOpType.subtract)
                        nc.sync.dma_start(out=out_d[nb0:nb1, :], in_=fin[:nn, :])

            for _rep in range(repeat):
                _phases()

    nc.compile()
    return nc


# ----------------------------------------------------------------------------
# driver
# ----------------------------------------------------------------------------

_CACHE = {}
_RUNNER = {}


def _build_runner(nc, n_cores):
    import jax
    from jax.sharding import Mesh, PartitionSpec
    from jax.experimental.shard_map import shard_map
    import concourse.mybir as mybir_
    from concourse import bass2jax
    from concourse.bass2jax import _bass_exec_p, partition_id_tensor

    bass2jax.install_neuronx_cc_hook()
    partition_name = (nc.partition_id_tensor.name
                      if nc.partition_id_tensor else None)
    in_names, out_names, out_avals, zero_outs = [], [], [], []
    for alloc in nc.m.functions[0].allocations:
        if not isinstance(alloc, mybir_.MemoryLocationSet):
            continue
        name = alloc.memorylocations[0].name
        if alloc.kind == "ExternalInput":
            if name != partition_name:
                in_names.append(name)
        elif alloc.kind == "ExternalOutput":
            out_names.append(name)
            shape = tuple(alloc.tensor_shape)
            dtype = mybir_.dt.np(alloc.dtype)
            out_avals.append(jax.core.ShapedArray(shape, dtype))
            zero_outs.append(np.zeros(shape, dtype))
    n_params = len(in_names)
    all_names = in_names + out_names
    if partition_name is not None:
        all_names.append(partition_name)

    def _body(*args):
        operands = list(args)
        if partition_name is not None:
            operands.append(partition_id_tensor())
        return tuple(_bass_exec_p.bind(
            *operands, out_avals=tuple(out_avals), in_names=tuple(all_names),
            out_names=tuple(out_names), lowering_input_output_aliases=(),
            sim_require_finite=True, sim_require_nnan=True, nc=nc))

    devices = jax.devices()[:n_cores]
    mesh = Mesh(np.asarray(devices), ("core",))
    n_out = len(out_names)
    fn = jax.jit(shard_map(_body, mesh=mesh,
                           in_specs=(PartitionSpec("core"),) * (n_params + n_out),
                           out_specs=(PartitionSpec("core"),) * n_out,
                           check_rep=False), keep_unused=True)
    return fn, in_names, out_names, out_avals, zero_outs, mesh


def _run_persistent(nc, in_maps, n_cores, key):
    import jax
    if key not in _RUNNER:
        fn, in_names, out_names, out_avals, zero_outs, mesh = \
            _build_runner(nc, n_cores)
        _RUNNER[key] = dict(fn=fn, in_names=in_names, out_names=out_names,
                            out_avals=out_avals, zero_outs=zero_outs,
                            mesh=mesh, dev_args=None)
    R = _RUNNER[key]
    concat_in = [np.concatenate([np.asarray(in_maps[c][nm])
                                 for c in range(n_cores)], axis=0)
                 for nm in R["in_names"]]
    concat_zero = [np.zeros((n_cores * z.shape[0], *z.shape[1:]), z.dtype)
                   for z in R["zero_outs"]]
    args = [jax.device_put(a) for a in concat_in + concat_zero]
    R["dev_args"] = args
    outs = R["fn"](*args)
    outs = [np.asarray(o) for o in outs]
    return {nm: outs[i].reshape(n_cores, *R["out_avals"][i].shape)
            for i, nm in enumerate(R["out_names"])}


def run(cfg, x, edge_index, is_reversed, weights, use_sim=False, repeat=1,
        layers=3, no_coll=False):
    lay, in_maps = host_prep(cfg, x, edge_index, is_reversed)
    wmap = host_prep_weights(cfg, **weights)
    for m in in_maps:
        m.update(wmap)

    sig = (lay.signature(), repeat, layers, no_coll)
    if sig in _CACHE:
        nc = _CACHE[sig]
    else:
        nc = build_program(cfg, lay, repeat=repeat, layers=layers,
                           no_coll=no_coll)
        _CACHE[sig] = nc

    n_cores = cfg["n_cores"]
    if use_sim:
        import concourse.bass_interp as bass_interp
        sim = bass_interp.MultiCoreSim(nc, n_cores, require_finite=False,
                                       require_nnan=False)
        for c in range(n_cores):
            for k, v in in_maps[c].items():
                sim.cores[c].tensor(k)[:] = v
        sim.simulate()
        outs = [np.array(sim.cores[c].tensor("out")) for c in range(n_cores)]
    else:
        key = str(sig)
        res = _run_persistent(nc, in_maps, n_cores, key)
        outs = list(res["out"])
    return np.concatenate(outs, axis=0)


def _marginal_ns(key, iters=6, reps=3):
    import jax, time as _t
    R = _RUNNER[key]
    fn, args = R["fn"], R["dev_args"]
    o = fn(*args); jax.block_until_ready(o)
    best = None
    for _ in range(reps):
        t0 = _t.time()
        o = fn(*args); jax.block_until_ready(o)
        base = _t.time() - t0
        t0 = _t.time()
        for _ in range(1 + iters):
            o = fn(*args)
        jax.block_until_ready(o)
        per = (_t.time() - t0 - base) / iters
        best = per if best is None else min(best, per)
    return best * 1e9


def time_device(inputs, iters=6, cfg=None, repeat_hi=4):
    """On-device execution time per kernel invocation: difference of
    in-program R-repeat vs 1-repeat marginal wall times cancels the
    fixed per-dispatch (axon RPC) overhead."""
    cfg = cfg or FULL_CFG
    weights = {k: np.asarray(inputs[k]) for k in
               ("W_st0", "b_st0", "W_ts0", "b_ts0", "W_st1", "b_st1",
                "W_ts1", "b_ts1", "W_last", "b_last")}
    keys = {}
    layers = int(__import__("os").environ.get("K2_LAYERS", "3"))
    no_coll = bool(int(__import__("os").environ.get("K2_NOCOLL", "0")))
    for rep in (1, repeat_hi):
        run(cfg, inputs["x"], inputs["edge_index"], inputs["is_reversed"],
            weights, repeat=rep, layers=layers, no_coll=no_coll)
        lay, _ = host_prep(cfg, inputs["x"], inputs["edge_index"],
                           inputs["is_reversed"])
        keys[rep] = str((lay.signature(), rep, layers, no_coll))
    m1 = _marginal_ns(keys[1], iters=iters)
    mR = _marginal_ns(keys[repeat_hi], iters=iters)
    return (mR - m1) / (repeat_hi - 1)


def kernel(x, edge_index, is_reversed, W_st0, b_st0, W_ts0, b_ts0,
           W_st1, b_st1, W_ts1, b_ts1, W_last, b_last):
    cfg = FULL_CFG
    weights = dict(W_st0=W_st0, b_st0=b_st0, W_ts0=W_ts0, b_ts0=b_ts0,
                   W_st1=W_st1, b_st1=b_st1, W_ts1=W_ts1, b_ts1=b_ts1,
                   W_last=W_last, b_last=b_last)
    out = run(cfg, x, edge_index, is_reversed, weights)
    return out.astype(np.float32)


# revision 12
# speedup vs baseline: 1.1271x; 1.1271x over previous
"""Trainium2 Bass kernel v2 for nn_BiModel (2-layer bidirectional GCN).

Distribution over 8 NeuronCores, nodes sharded 6250/core.

Structure (vs v1):
- Each layer AllGathers its bf16 message table in two per-core half
  slices so the second collective overlaps with aggregation of the first
  half; gathers read the collective output directly (no DRAM copy).
- Node halves are block-aligned and padded: half 0 = local rows [0,3200),
  half 1 = [3200,6250) padded to 3072 rows.  Half tables hold
  8*3200=25600 / 8*3072=24576 rows (int16-gatherable).
- Edge chunks are capacity-padded per (branch, src-half, dst-block); both
  branches' one-hot matmuls pair into a single PSUM tile per dst block.
- Layer 2 aggregates prescaled h2 directly; W_last applied afterwards.
"""

import numpy as np

import concourse.bass as bass
import concourse.bacc as bacc
import concourse.mybir as mybir
import concourse.tile as tile
from concourse.bass_utils import run_bass_kernel_spmd
from concourse.masks import make_identity

import ml_dtypes

P = 128
F32 = mybir.dt.float32
BF16 = mybir.dt.bfloat16
I16 = mybir.dt.int16
I32 = mybir.dt.int32

FULL_CFG = dict(n=50000, e=800000, f_in=500, h=64, c_out=16, n_cores=8,
                cap_floor=5, blocks_per_group=5, hl=3200)


def cdiv(a, b):
    return (a + b - 1) // b


# ----------------------------------------------------------------------------
# host-side layout / preprocessing
# ----------------------------------------------------------------------------

class Layout2:
    """Compile-time layout shared by all cores (uniform SPMD program).
    Edge chunk capacity per (branch, src-half, dst-block) = max count over
    cores rounded up to 128 chunks, floored at cap_floor chunks."""

    def __init__(self, cfg, counts):
        # counts: [n_cores, 2, 2, nblk] (core, branch, src-half, dst-blk)
        self.cfg = cfg
        self.nloc = cfg["n"] // cfg["n_cores"]
        self.nblk = cdiv(self.nloc, P)
        hl = cfg["hl"]
        self.hpad = [hl, self.nblk * P - hl]
        self.htot = [hp * cfg["n_cores"] for hp in self.hpad]
        cap = counts.max(axis=0)                      # [2, 2, nblk]
        self.cap_chunks = np.maximum(cdiv(cap, P), cfg["cap_floor"])
        self.chunk_off = np.zeros((2, 2, self.nblk), np.int64)
        self.nchunks_bh = np.zeros((2, 2), np.int64)
        for b in range(2):
            for h in range(2):
                off = 0
                for blk in range(self.nblk):
                    self.chunk_off[b, h, blk] = off
                    off += self.cap_chunks[b, h, blk]
                self.nchunks_bh[b, h] = off
        bg = cfg["blocks_per_group"]
        self.groups = [list(range(g * bg, min((g + 1) * bg, self.nblk)))
                       for g in range(cdiv(self.nblk, bg))]

    def signature(self):
        return (tuple(self.cap_chunks.reshape(-1).tolist()),
                tuple(sorted(self.cfg.items())))


def _wrap_idx16(idx, n_pad):
    buf = np.zeros(n_pad, np.int16)
    buf[: len(idx)] = idx.astype(np.int16)
    w = buf.reshape(n_pad // 16, 16).T            # [16, n/16]
    return np.ascontiguousarray(np.tile(w, (8, 1)))  # [128, n/16]


def host_prep(cfg, x, edge_index, is_reversed):
    n, f_in = cfg["n"], cfg["f_in"]
    n_cores = cfg["n_cores"]
    nloc = n // n_cores
    nblk = cdiv(nloc, P)
    f_pad = cdiv(f_in, P) * P
    kch = f_pad // P
    hl = cfg["hl"]
    hpad = [hl, nblk * P - hl]

    src = np.asarray(edge_index[0], np.int64)
    dst = np.asarray(edge_index[1], np.int64)
    rev = np.asarray(is_reversed).astype(bool)

    core = dst // nloc
    dl = dst % nloc
    blk = dl // P
    branch = rev.astype(np.int64)
    cs = src // nloc
    rs = src % nloc
    hf = (rs >= hl).astype(np.int64)               # src half
    tblidx = cs * np.where(hf == 0, hpad[0], hpad[1]) + (rs - hf * hl)

    key = (((core * 2 + branch) * 2 + hf) * nblk) + blk
    order = np.argsort(key, kind="stable")
    counts = np.bincount(key[order], minlength=n_cores * 2 * 2 * nblk)
    counts = counts.reshape(n_cores, 2, 2, nblk)
    lay = Layout2(cfg, counts)

    deg = np.zeros((2, n), np.float32)
    np.add.at(deg[0], dst[~rev], 1.0)
    np.add.at(deg[1], dst[rev], 1.0)

    # node-feature transpose, bf16, padded
    xT = np.zeros((f_pad, n), ml_dtypes.bfloat16)
    xT[:f_in] = np.asarray(x, np.float32).T

    tbl_s = tblidx[order]
    dl_s = dl[order]
    gs = np.concatenate([[0], np.cumsum(counts.reshape(-1))])[:-1]
    gs = gs.reshape(n_cores, 2, 2, nblk)

    nblk_pad = nblk * P
    in_maps = []
    for c in range(n_cores):
        xc = xT[:, c * nloc:(c + 1) * nloc].reshape(kch, P, nloc)
        m = {"xT": np.ascontiguousarray(
            xc.transpose(1, 0, 2).reshape(P, kch * nloc))}
        degs = np.ones((P, 2 * nblk), np.float32)
        for b in range(2):
            dloc = np.ones(nblk_pad, np.float32)
            dloc[:nloc] = deg[b, c * nloc:(c + 1) * nloc]
            degs[:, b * nblk:(b + 1) * nblk] = dloc.reshape(nblk, P).T
        m["degs"] = degs
        for b in range(2):
            for h in range(2):
                nch = max(int(lay.nchunks_bh[b, h]), 1)
                tot = nch * P
                idx_stream = np.zeros(tot, np.int16)
                dstv = np.full(tot, -1.0, np.float32)
                for blk_ in range(nblk):
                    cnt = int(counts[c, b, h, blk_])
                    s0 = int(gs[c, b, h, blk_])
                    co = int(lay.chunk_off[b, h, blk_]) * P
                    idx_stream[co:co + cnt] = tbl_s[s0:s0 + cnt]
                    dstv[co:co + cnt] = dl_s[s0:s0 + cnt] - blk_ * P
                m[f"idx_b{b}h{h}"] = _wrap_idx16(idx_stream, tot)
                m[f"dst_b{b}h{h}"] = np.ascontiguousarray(
                    dstv.reshape(nch, P).T)          # [128, nch]
        in_maps.append(m)
    return lay, in_maps


def host_prep_weights(cfg, W_st0, b_st0, W_ts0, b_ts0, W_st1, b_st1,
                      W_ts1, b_ts1, W_last, b_last):
    f_in, h, c_out = cfg["f_in"], cfg["h"], cfg["c_out"]
    f_pad = cdiv(f_in, P) * P
    W0 = np.zeros((f_pad, 2 * h), np.float32)
    W0[:f_in, :h] = W_st0
    W0[:f_in, h:] = W_ts0
    kch = f_pad // P
    W0 = np.ascontiguousarray(
        W0.reshape(kch, P, 2 * h).transpose(1, 0, 2).reshape(P, kch * 2 * h))
    W1 = np.concatenate([W_st1, W_ts1], axis=1).astype(np.float32)
    WL = np.zeros((2 * h, 128), np.float32)
    WL[:, :c_out] = W_last
    bias01 = np.stack([np.concatenate([b_st0, b_ts0]),
                       np.concatenate([b_st1, b_ts1])], axis=1).astype(np.float32)
    return dict(W0=W0.astype(ml_dtypes.bfloat16),
                W1=W1.astype(ml_dtypes.bfloat16),
                WL=WL.astype(ml_dtypes.bfloat16), WLf=WL, bias01=bias01,
                b_last=np.asarray(b_last, np.float32).reshape(c_out, 1))


# ----------------------------------------------------------------------------
# device program
# ----------------------------------------------------------------------------

def build_program(cfg, lay, repeat=1, layers=3, no_coll=False):
    n, f_in = cfg["n"], cfg["f_in"]
    h, c_out = cfg["h"], cfg["c_out"]
    n_cores = cfg["n_cores"]
    nloc = n // n_cores
    nblk = lay.nblk
    nblk_pad = nblk * P
    f_pad = cdiv(f_in, P) * P
    kch = f_pad // P
    h2 = 2 * h
    hpad = lay.hpad
    htot = lay.htot
    hblk = [hpad[0] // P, hpad[1] // P]           # local blocks per half
    core_ids = list(range(n_cores))

    nc = bacc.Bacc("TRN2", target_bir_lowering=False, debug=False,
                   num_devices=n_cores)

    xT_d = nc.declare_dram_parameter("xT", [P, kch * nloc], BF16, isOutput=False)
    degs_d = nc.declare_dram_parameter("degs", [P, 2 * nblk], F32, isOutput=False)
    W0_d = nc.declare_dram_parameter("W0", [P, kch * h2], BF16, isOutput=False)
    W1_d = nc.declare_dram_parameter("W1", [h2, h2], BF16, isOutput=False)
    WL_d = nc.declare_dram_parameter("WL", [h2, 128], BF16, isOutput=False)
    WLf_d = nc.declare_dram_parameter("WLf", [h2, 128], F32, isOutput=False)
    bias01_d = nc.declare_dram_parameter("bias01", [h2, 2], F32, isOutput=False)
    b_last_d = nc.declare_dram_parameter("b_last", [c_out, 1], F32, isOutput=False)
    idx_d, dst_d = {}, {}
    for b in range(2):
        for hf in range(2):
            ncw = max(int(lay.nchunks_bh[b, hf]), 1)
            idx_d[b, hf] = nc.declare_dram_parameter(
                f"idx_b{b}h{hf}", [P, ncw * 8], I16, isOutput=False)
            dst_d[b, hf] = nc.declare_dram_parameter(
                f"dst_b{b}h{hf}", [P, ncw], F32, isOutput=False)
    out_d = nc.declare_dram_parameter("out", [nloc, c_out], F32, isOutput=True)

    tbl_loc_h = [nc.dram_tensor(f"tblloc_h{i}", [hpad[i], h2], BF16)
                 for i in range(2)]
    tbl_half = [nc.dram_tensor(f"tbl_h{i}", [htot[i], h2], BF16,
                               addr_space="Shared") for i in range(2)]

    with tile.TileContext(nc) as tc:
        with (
            tc.tile_pool(name="persist", bufs=1) as pp,
            tc.tile_pool(name="init", bufs=1) as ip,
            tc.tile_pool(name="work", bufs=2) as wp,
            tc.tile_pool(name="xload", bufs=3) as xp,
            tc.tile_pool(name="msg", bufs=2) as mp,
            tc.tile_pool(name="mask", bufs=6) as kp,
            tc.tile_pool(name="psA", bufs=3, space="PSUM") as psA,
            tc.tile_pool(name="psN", bufs=2, space="PSUM") as psN,
            tc.tile_pool(name="psB", bufs=2, space="PSUM") as psB,
        ):
            # ---------- constants ----------
            iota_i = ip.tile([P, P], I32, tag="ioi")
            nc.gpsimd.iota(iota_i[:], pattern=[[1, P]], base=0,
                           channel_multiplier=0)
            iota_bf = pp.tile([P, P], BF16, tag="iobf")
            nc.vector.tensor_copy(iota_bf[:], iota_i[:])
            ident = pp.tile([P, P], F32, tag="ident")
            make_identity(nc, ident[:])
            ident_bf = pp.tile([P, P], BF16, tag="identbf")
            nc.vector.tensor_copy(ident_bf[:], ident[:])
            biasv = pp.tile([P, 2], F32, tag="biasv")
            nc.sync.dma_start(out=biasv[:], in_=bias01_d[:, :])
            biasL = pp.tile([c_out, 1], F32, tag="biasL")
            nc.sync.dma_start(out=biasL[:], in_=b_last_d[:, :])

            # ---------- local degrees -> dinv [128, 3*nblk] (st|ts|all) ----
            deg_sb = ip.tile([P, 2 * nblk], F32, tag="degsb")
            nc.sync.dma_start(out=deg_sb[:], in_=degs_d[:, :])
            dtmp = ip.tile([P, 3 * nblk], F32, tag="dtmp")
            nc.vector.tensor_tensor(out=dtmp[:, 2 * nblk:],
                                    in0=deg_sb[:, :nblk], in1=deg_sb[:, nblk:],
                                    op=mybir.AluOpType.add)
            nc.vector.tensor_copy(dtmp[:, :2 * nblk], deg_sb[:])
            nc.vector.tensor_scalar_add(dtmp[:], dtmp[:], 1.0)
            dsq = ip.tile([P, 3 * nblk], F32, tag="dsq")
            nc.scalar.sqrt(dsq[:], dtmp[:])
            dinv = pp.tile([P, 3 * nblk], F32, tag="dinv")
            nc.vector.reciprocal(dinv[:], dsq[:])


            # dinv broadcast rows [128, nblk_pad] (st rows 0:64, ts 64:128)
            # and [c_out, nblk_pad] of dinv_all
            dinvT = ip.tile([nblk, 3 * P], F32, tag="dinvT")
            for i in range(3):
                tps = psB.tile([nblk, P], F32, tag="pst")
                nc.tensor.transpose(tps[:], dinv[:, i * nblk:(i + 1) * nblk],
                                    ident[:])
                nc.scalar.copy(dinvT[:, i * P:(i + 1) * P], tps[:])
            dinv_flat_d = nc.dram_tensor("dinv_flat", [1, 3 * nblk_pad], F32)
            for i in range(3):
                nc.sync.dma_start(
                    out=dinv_flat_d[0:1, i * nblk_pad:(i + 1) * nblk_pad],
                    in_=dinvT[:, i * P:(i + 1) * P])
            ones_row = pp.tile([1, P], F32, tag="ones_row")
            nc.vector.memset(ones_row[:], 1.0)
            dinvb = pp.tile([P, nblk_pad], F32, tag="dinvb")
            dinvallb = pp.tile([c_out, nblk_pad], F32, tag="dinvallb")
            NTB = 512
            for t0 in range(0, nblk_pad, NTB):
                t1 = min(t0 + NTB, nblk_pad)
                dfs = ip.tile([1, 3 * NTB], F32, tag="dfs")
                for i in range(3):
                    nc.sync.dma_start(
                        out=dfs[0:1, i * NTB: i * NTB + t1 - t0],
                        in_=dinv_flat_d[0:1, i * nblk_pad + t0: i * nblk_pad + t1])
                bps = psB.tile([P, NTB], F32, tag="pst")
                nc.tensor.matmul(bps[0:h, :t1 - t0], lhsT=ones_row[0:1, 0:h],
                                 rhs=dfs[0:1, 0:t1 - t0],
                                 start=True, stop=True)
                nc.tensor.matmul(bps[h:h2, :t1 - t0], lhsT=ones_row[0:1, 0:h],
                                 rhs=dfs[0:1, NTB:NTB + t1 - t0],
                                 start=True, stop=True, tile_position=(0, h))
                nc.scalar.copy(dinvb[:, t0:t1], bps[:, :t1 - t0])
                bps2 = psB.tile([P, NTB], F32, tag="pst")
                nc.tensor.matmul(bps2[:c_out, :t1 - t0],
                                 lhsT=ones_row[0:1, 0:c_out],
                                 rhs=dfs[0:1, 2 * NTB:2 * NTB + t1 - t0],
                                 start=True, stop=True)
                nc.scalar.copy(dinvallb[:, t0:t1], bps2[:c_out, :t1 - t0])

            # ---------- weights ----------
            w0_sb = pp.tile([P, kch * h2], BF16, tag="w0")
            nc.sync.dma_start(out=w0_sb[:], in_=W0_d[:, :])
            w1_sb = pp.tile([P, h2], BF16, tag="w1")
            nc.sync.dma_start(out=w1_sb[:], in_=W1_d[:, :])
            wl_sb = pp.tile([P, 128], BF16, tag="wl")
            nc.sync.dma_start(out=wl_sb[:], in_=WL_d[:, :])
            wlf_sb = pp.tile([P, 128], F32, tag="wlf")
            nc.sync.dma_start(out=wlf_sb[:], in_=WLf_d[:, :])

            # ---------- state ----------
            hT = pp.tile([P, nblk_pad], BF16, tag="hT")
            h2T = pp.tile([P, nblk_pad], BF16, tag="h2T")
            xwT = pp.tile([P, nblk_pad], F32, tag="xwT")
            aggT = pp.tile([P, nblk_pad], F32, tag="aggT")
            xwTL = xwT[0:c_out, :]
            outTL = aggT[0:c_out, :]

            # ---------------------------------------------------------------
            def build_xw_featmajor(src_getter, src_kch, w_ap_of_k, rows, dst):
                NT = 512
                for t0 in range(0, nloc, NT):
                    t1 = min(t0 + NT, nloc)
                    ps = psB.tile([P, NT], F32, tag="pst")
                    for k in range(src_kch):
                        nc.tensor.matmul(
                            ps[:rows, :t1 - t0],
                            lhsT=w_ap_of_k(k)[:, :rows],
                            rhs=src_getter(k, t0, t1),
                            start=(k == 0), stop=(k == src_kch - 1))
                    nc.scalar.copy(dst[:rows, t0:t1], ps[:rows, :t1 - t0])

            def build_table_rows(src_ap, w_ap, tbl_dst, blk_lo, blk_hi,
                                 row_base, prescale):
                """node-major prescaled bf16 table rows from feature-major
                SBUF source (single k chunk); all blocks full (padded)."""
                for blk in range(blk_lo, blk_hi):
                    nb0 = blk * P
                    ps = psN.tile([P, h2], F32, tag="psnm")
                    nc.tensor.matmul(ps[:, :], lhsT=src_ap[:, nb0:nb0 + P],
                                     rhs=w_ap, start=True, stop=True)
                    tt = wp.tile([P, h2], BF16, tag="tblt")
                    for (c0, c1, dcol) in prescale:
                        nc.vector.tensor_scalar_mul(
                            tt[:, c0:c1], ps[:, c0:c1],
                            dinv[:, dcol * nblk + blk: dcol * nblk + blk + 1])
                    nc.sync.dma_start(
                        out=tbl_dst[nb0 - row_base:nb0 - row_base + P, 0:h2],
                        in_=tt[:, :])

            # ---------------------------------------------------------------
            def aggregate(tables, full_rows):
                """Gather + one-hot-matmul segment sums over the 4 edge
                streams (branch x src-half).  full_rows=False: branch b uses
                lhs cols/psum rows [b*h,(b+1)*h).  full_rows=True (layer 2):
                full 128-wide lhs, both branches accumulate into all rows."""
                nmask = [0]

                def build_mask(dstcol_ap):
                    mk = kp.tile([P, P], BF16, tag="mask")
                    eng = nc.vector if nmask[0] % 3 else nc.gpsimd
                    nmask[0] += 1
                    eng.tensor_scalar(
                        out=mk[:], in0=iota_bf[:], scalar1=dstcol_ap,
                        scalar2=None, op0=mybir.AluOpType.is_equal)
                    return mk

                for hf in range(2):
                    for blocks in lay.groups:
                        bufs, dls = {}, {}
                        for b in range(2):
                            ch0 = int(lay.chunk_off[b, hf, blocks[0]])
                            ch1 = int(lay.chunk_off[b, hf, blocks[-1]]
                                      + lay.cap_chunks[b, hf, blocks[-1]])
                            nch = ch1 - ch0
                            it = wp.tile([P, nch * 8], I16, tag=f"idx{b}")
                            nc.sync.dma_start(
                                out=it[:], in_=idx_d[b, hf][:, ch0 * 8: ch1 * 8])
                            dt = wp.tile([P, nch], F32, tag=f"dl{b}")
                            nc.sync.dma_start(
                                out=dt[:], in_=dst_d[b, hf][:, ch0:ch1])
                            buf = mp.tile([P, nch, h2], BF16, tag=f"msg{b}")
                            nidx = nch * P
                            nc.gpsimd.dma_gather(
                                out_ap=buf[:], in_ap=tables[hf][:, :],
                                idxs_ap=it[:], num_idxs=nidx,
                                num_idxs_reg=nidx, elem_size=h2,
                                single_packet=(nidx <= 1024))
                            bufs[b] = (buf, ch0)
                            dls[b] = (dt, ch0)
                        for blk in blocks:
                            nb = slice(blk * P, (blk + 1) * P)
                            ps = psA.tile([P, P], F32, tag="agg")
                            for b in range(2):
                                buf, ch0 = bufs[b]
                                dt, _ = dls[b]
                                ncap = int(lay.cap_chunks[b, hf, blk])
                                co = int(lay.chunk_off[b, hf, blk])
                                for j in range(ncap):
                                    mk = build_mask(
                                        dt[:, co - ch0 + j: co - ch0 + j + 1])
                                    if full_rows:
                                        lh = buf[:, co - ch0 + j, 0:h2]
                                        o = ps[:, :]
                                        tpos = None
                                        st = (b == 0 and j == 0)
                                        sp = (b == 1 and j == ncap - 1)
                                    else:
                                        lh = buf[:, co - ch0 + j,
                                                 b * h:(b + 1) * h]
                                        o = ps[b * h:(b + 1) * h, :]
                                        tpos = (0, b * h)
                                        st = (j == 0)
                                        sp = (j == ncap - 1)
                                    nc.tensor.matmul(o, lhsT=lh, rhs=mk[:],
                                                     start=st, stop=sp,
                                                     tile_position=tpos)
                            if hf == 0:
                                nc.scalar.copy(aggT[:, nb], ps[:, :])
                            else:
                                nc.vector.tensor_add(out=aggT[:, nb],
                                                     in0=aggT[:, nb],
                                                     in1=ps[:, :])

            # ---------------------------------------------------------------
            def post01(layer, out_tile):
                nc.vector.tensor_tensor(out=xwT[:, :], in0=xwT[:, :],
                                        in1=dinvb[:, :],
                                        op=mybir.AluOpType.mult)
                nc.vector.tensor_tensor(out=aggT[:, :], in0=aggT[:, :],
                                        in1=xwT[:, :],
                                        op=mybir.AluOpType.add)
                nc.vector.tensor_tensor(out=aggT[:, :], in0=aggT[:, :],
                                        in1=dinvb[:, :],
                                        op=mybir.AluOpType.mult)
                nc.scalar.activation(out_tile[:, :], aggT[:, :],
                                     mybir.ActivationFunctionType.Relu,
                                     bias=biasv[:, layer:layer + 1])

            # ---------------------------------------------------------------
            def _phases():
                if nblk_pad > nloc:
                    nc.vector.memset(xwT[:, nloc:], 0.0)
                    nc.vector.memset(hT[:, nloc:], 0.0)
                    nc.vector.memset(h2T[:, nloc:], 0.0)

                # =================== layer 0 ===================
                with nc.named_scope("L0_tables"):
                    # zero padded tail rows of the half-1 local table once
                    zpad = hpad[1] * P - (nloc - hpad[0] * 1)
                    padrows = hpad[0] + hpad[1] - nloc      # 22
                    if padrows > 0:
                        zt = wp.tile([padrows, h2], BF16, tag="zpad")
                        nc.vector.memset(zt[:], 0.0)
                        nc.sync.dma_start(
                            out=tbl_loc_h[1][hpad[1] - padrows:hpad[1], :],
                            in_=zt[:])
                    NT = 512
                    for t0 in range(0, nloc, NT):
                        t1 = min(t0 + NT, nloc)
                        xts = []
                        for k in range(kch):
                            t = xp.tile([P, NT], BF16, tag=f"xb{k}")
                            nc.sync.dma_start(
                                out=t[:, :t1 - t0],
                                in_=xT_d[:, k * nloc + t0: k * nloc + t1])
                            xts.append(t)
                        # feature-major xw for the self-loop term
                        ps = psB.tile([P, NT], F32, tag="pst")
                        for k in range(kch):
                            nc.tensor.matmul(
                                ps[:h2, :t1 - t0],
                                lhsT=w0_sb[:, k * h2:(k + 1) * h2],
                                rhs=xts[k][:, :t1 - t0],
                                start=(k == 0), stop=(k == kch - 1))
                        nc.scalar.copy(xwT[:h2, t0:t1], ps[:h2, :t1 - t0])
                        # node-major prescaled table rows
                        for blk in range(t0 // P, cdiv(t1, P)):
                            nb0 = blk * P
                            nn = min(P, nloc - nb0)
                            psn = psN.tile([P, h2], F32, tag="psnm")
                            for k in range(kch):
                                nc.tensor.matmul(
                                    psn[:nn, :],
                                    lhsT=xts[k][:, nb0 - t0:nb0 - t0 + nn],
                                    rhs=w0_sb[:, k * h2:(k + 1) * h2],
                                    start=(k == 0), stop=(k == kch - 1))
                            tt = wp.tile([P, h2], BF16, tag="tblt")
                            for (cc0, cc1, dcol) in ((0, h, 0), (h, h2, 1)):
                                nc.vector.tensor_scalar_mul(
                                    tt[:nn, cc0:cc1], psn[:nn, cc0:cc1],
                                    dinv[:nn, dcol * nblk + blk:
                                         dcol * nblk + blk + 1])
                            hf = 0 if blk < hblk[0] else 1
                            r0 = nb0 - (hpad[0] if hf else 0)
                            nc.sync.dma_start(
                                out=tbl_loc_h[hf][r0:r0 + nn, :],
                                in_=tt[:nn, :])
                    if not no_coll:
                        for hf in range(2):
                            nc.gpsimd.collective_compute(
                                "AllGather", mybir.AluOpType.bypass,
                                replica_groups=[core_ids],
                                ins=[tbl_loc_h[hf][:]], outs=[tbl_half[hf][:]])
                with nc.named_scope("L0_agg"):
                    aggregate(tbl_half, False)
                with nc.named_scope("L0_post"):
                    post01(0, hT)
                if layers <= 1:
                    nc.sync.dma_start(out=out_d[0:P, :], in_=xwT[0:P, 0:c_out])
                    return

                # =================== layer 1 ===================
                with nc.named_scope("L1_tables"):
                    for hf in range(2):
                        build_table_rows(hT, w1_sb[:], tbl_loc_h[hf],
                                         0 if hf == 0 else hblk[0],
                                         hblk[0] if hf == 0 else nblk,
                                         0 if hf == 0 else hpad[0],
                                         ((0, h, 0), (h, h2, 1)))
                        if not no_coll:
                            nc.gpsimd.collective_compute(
                                "AllGather", mybir.AluOpType.bypass,
                                replica_groups=[core_ids],
                                ins=[tbl_loc_h[hf][:]], outs=[tbl_half[hf][:]])
                    build_xw_featmajor(lambda k, a, bb: hT[:, a:bb], 1,
                                       lambda k: w1_sb[:], h2, xwT)
                with nc.named_scope("L1_agg"):
                    aggregate(tbl_half, False)
                with nc.named_scope("L1_post"):
                    post01(1, h2T)
                if layers <= 2:
                    nc.sync.dma_start(out=out_d[0:P, :], in_=xwT[0:P, 0:c_out])
                    return

                # =================== layer 2 ===================
                with nc.named_scope("L2_tables"):
                    for hf in range(2):
                        build_table_rows(h2T, ident_bf[:], tbl_loc_h[hf],
                                         0 if hf == 0 else hblk[0],
                                         hblk[0] if hf == 0 else nblk,
                                         0 if hf == 0 else hpad[0],
                                         ((0, h2, 2),))
                        if not no_coll:
                            nc.gpsimd.collective_compute(
                                "AllGather", mybir.AluOpType.bypass,
                                replica_groups=[core_ids],
                                ins=[tbl_loc_h[hf][:]], outs=[tbl_half[hf][:]])
                    build_xw_featmajor(lambda k, a, bb: h2T[:, a:bb], 1,
                                       lambda k: wl_sb[:], c_out, xwT)
                with nc.named_scope("L2_agg"):
                    aggregate(tbl_half, True)

                # out16 = (WL^T aggT)*dinvall + xwTL*dinvall^2 + b_last
                with nc.named_scope("L2_post"):
                    NT = 512
                    for t0 in range(0, nblk_pad, NT):
                        t1 = min(t0 + NT, nblk_pad)
                        ps = psB.tile([P, NT], F32, tag="pst")
                        nc.tensor.matmul(ps[:c_out, :t1 - t0],
                                         lhsT=wlf_sb[:, :c_out],
                                         rhs=aggT[:, t0:t1],
                                         start=True, stop=True)
                        nc.scalar.copy(outTL[:, t0:t1], ps[:c_out, :t1 - t0])
                    nc.vector.tensor_tensor(out=xwTL[:, :], in0=xwTL[:, :],
                                            in1=dinvallb[:, :],
                                            op=mybir.AluOpType.mult)
                    nc.vector.tensor_tensor(out=outTL[:, :], in0=outTL[:, :],
                                            in1=xwTL[:, :],
                                            op=mybir.AluOpType.add)
                    nc.vector.tensor_tensor(out=outTL[:, :], in0=outTL[:, :],
                                            in1=dinvallb[:, :],
                                            op=mybir.AluOpType.mult)
                    nc.scalar.activation(outTL[:, :], outTL[:, :],
                                         mybir.ActivationFunctionType.Identity,
                                         bias=biasL[:, 0:1])

                with nc.named_scope("softmax"):
                    for blk in range(nblk):
                        nb0 = blk * P
                        nb1 = min(nb0 + P, nloc)
                        nn = nb1 - nb0
                        if nn <= 0:
                            continue
                        tp = psB.tile([P, c_out], F32, tag="pst")
                        nc.tensor.transpose(tp[:], outTL[:, nb0:nb0 + P],
                                            ident[:c_out, :c_out])
                        negmax = wp.tile([P, 1], F32, tag="negmax")
                        nc.vector.tensor_reduce(negmax[:], tp[:],
                                                axis=mybir.AxisListType.X,
                                                op=mybir.AluOpType.max,
                                                negate=True)
                        ex = wp.tile([P, c_out], F32, tag="ex")
                        nc.scalar.activation(ex[:], tp[:],
                                             mybir.ActivationFunctionType.Exp,
                                             bias=negmax[:, 0:1])
                        sume = wp.tile([P, 1], F32, tag="sume")
                        nc.vector.tensor_reduce(sume[:], ex[:],
                                                axis=mybir.AxisListType.X,
                                                op=mybir.AluOpType.add)
                        lse = wp.tile([P, 1], F32, tag="lse")
                        nc.scalar.activation(lse[:], sume[:],
                                             mybir.ActivationFunctionType.Ln)
                        fin = wp.tile([P, c_out], F32, tag="fin")
                        nc.vector.tensor_scalar(
                            out=fin[:], in0=tp[:], scalar1=negmax[:, 0:1],
                            scalar2=lse[:, 0:1], op0=mybir.AluOpType.add,
                            op1=mybir.AluOpType.subtract)
                        nc.sync.dma_start(out=out_d[nb0:nb1, :], in_=fin[:nn, :])

            for _rep in range(repeat):
                _phases()

    nc.compile()
    return nc


# ----------------------------------------------------------------------------
# driver
# ----------------------------------------------------------------------------

_CACHE = {}
_RUNNER = {}


def _build_runner(nc, n_cores):
    import jax
    from jax.sharding import Mesh, PartitionSpec
    from jax.experimental.shard_map import shard_map
    import concourse.mybir as mybir_
    from concourse import bass2jax
    from concourse.bass2jax import _bass_exec_p, partition_id_tensor

    bass2jax.install_neuronx_cc_hook()
    partition_name = (nc.partition_id_tensor.name
                      if nc.partition_id_tensor else None)
    in_names, out_names, out_avals, zero_outs = [], [], [], []
    for alloc in nc.m.functions[0].allocations:
        if not isinstance(alloc, mybir_.MemoryLocationSet):
            continue
        name = alloc.memorylocations[0].name
        if alloc.kind == "ExternalInput":
            if name != partition_name:
                in_names.append(name)
        elif alloc.kind == "ExternalOutput":
            out_names.append(name)
            shape = tuple(alloc.tensor_shape)
            dtype = mybir_.dt.np(alloc.dtype)
            out_avals.append(jax.core.ShapedArray(shape, dtype))
            zero_outs.append(np.zeros(shape, dtype))
    n_params = len(in_names)
    all_names = in_names + out_names
    if partition_name is not None:
        all_names.append(partition_name)

    def _body(*args):
        operands = list(args)
        if partition_name is not None:
            operands.append(partition_id_tensor())
        return tuple(_bass_exec_p.bind(
            *operands, out_avals=tuple(out_avals), in_names=tuple(all_names),
            out_names=tuple(out_names), lowering_input_output_aliases=(),
            sim_require_finite=True, sim_require_nnan=True, nc=nc))

    devices = jax.devices()[:n_cores]
    mesh = Mesh(np.asarray(devices), ("core",))
    n_out = len(out_names)
    fn = jax.jit(shard_map(_body, mesh=mesh,
                           in_specs=(PartitionSpec("core"),) * (n_params + n_out),
                           out_specs=(PartitionSpec("core"),) * n_out,
                           check_rep=False), keep_unused=True)
    return fn, in_names, out_names, out_avals, zero_outs, mesh


def _run_persistent(nc, in_maps, n_cores, key):
    import jax
    if key not in _RUNNER:
        fn, in_names, out_names, out_avals, zero_outs, mesh = \
            _build_runner(nc, n_cores)
        _RUNNER[key] = dict(fn=fn, in_names=in_names, out_names=out_names,
                            out_avals=out_avals, zero_outs=zero_outs,
                            mesh=mesh, dev_args=None)
    R = _RUNNER[key]
    concat_in = [np.concatenate([np.asarray(in_maps[c][nm])
                                 for c in range(n_cores)], axis=0)
                 for nm in R["in_names"]]
    concat_zero = [np.zeros((n_cores * z.shape[0], *z.shape[1:]), z.dtype)
                   for z in R["zero_outs"]]
    args = [jax.device_put(a) for a in concat_in + concat_zero]
    R["dev_args"] = args
    outs = R["fn"](*args)
    outs = [np.asarray(o) for o in outs]
    return {nm: outs[i].reshape(n_cores, *R["out_avals"][i].shape)
            for i, nm in enumerate(R["out_names"])}


def run(cfg, x, edge_index, is_reversed, weights, use_sim=False, repeat=1,
        layers=3, no_coll=False):
    lay, in_maps = host_prep(cfg, x, edge_index, is_reversed)
    wmap = host_prep_weights(cfg, **weights)
    for m in in_maps:
        m.update(wmap)

    sig = (lay.signature(), repeat, layers, no_coll)
    if sig in _CACHE:
        nc = _CACHE[sig]
    else:
        nc = build_program(cfg, lay, repeat=repeat, layers=layers,
                           no_coll=no_coll)
        _CACHE[sig] = nc

    n_cores = cfg["n_cores"]
    if use_sim:
        import concourse.bass_interp as bass_interp
        sim = bass_interp.MultiCoreSim(nc, n_cores, require_finite=False,
                                       require_nnan=False)
        for c in range(n_cores):
            for k, v in in_maps[c].items():
                sim.cores[c].tensor(k)[:] = v
        sim.simulate()
        outs = [np.array(sim.cores[c].tensor("out")) for c in range(n_cores)]
    else:
        key = str(sig)
        res = _run_persistent(nc, in_maps, n_cores, key)
        outs = list(res["out"])
    return np.concatenate(outs, axis=0)


def _marginal_sample(fn, args, iters):
    import jax, time as _t
    t0 = _t.time()
    o = fn(*args); jax.block_until_ready(o)
    base = _t.time() - t0
    t0 = _t.time()
    for _ in range(1 + iters):
        o = fn(*args)
    jax.block_until_ready(o)
    return (_t.time() - t0 - base) / iters * 1e9


def _marginal_ns(key, iters=6, reps=3):
    import jax
    R = _RUNNER[key]
    fn, args = R["fn"], R["dev_args"]
    o = fn(*args); jax.block_until_ready(o)
    return min(_marginal_sample(fn, args, iters) for _ in range(reps))


def time_device(inputs, iters=6, cfg=None, repeat_hi=4):
    """On-device execution time per kernel invocation: difference of
    in-program R-repeat vs 1-repeat marginal wall times cancels the
    fixed per-dispatch (axon RPC) overhead."""
    cfg = cfg or FULL_CFG
    weights = {k: np.asarray(inputs[k]) for k in
               ("W_st0", "b_st0", "W_ts0", "b_ts0", "W_st1", "b_st1",
                "W_ts1", "b_ts1", "W_last", "b_last")}
    keys = {}
    layers = int(__import__("os").environ.get("K2_LAYERS", "3"))
    no_coll = bool(int(__import__("os").environ.get("K2_NOCOLL", "0")))
    for rep in (1, repeat_hi):
        run(cfg, inputs["x"], inputs["edge_index"], inputs["is_reversed"],
            weights, repeat=rep, layers=layers, no_coll=no_coll)
        lay, _ = host_prep(cfg, inputs["x"], inputs["edge_index"],
                           inputs["is_reversed"])
        keys[rep] = str((lay.signature(), rep, layers, no_coll))
    import jax
    R1, RH = _RUNNER[keys[1]], _RUNNER[keys[repeat_hi]]
    for R in (R1, RH):
        o = R["fn"](*R["dev_args"]); jax.block_until_ready(o)
    m1s, mRs = [], []
    for _ in range(5):
        m1s.append(_marginal_sample(R1["fn"], R1["dev_args"], max(iters, 8)))
        mRs.append(_marginal_sample(RH["fn"], RH["dev_args"], max(iters, 8)))
    return (min(mRs) - min(m1s)) / (repeat_hi - 1)


def kernel(x, edge_index, is_reversed, W_st0, b_st0, W_ts0, b_ts0,
           W_st1, b_st1, W_ts1, b_ts1, W_last, b_last):
    cfg = FULL_CFG
    weights = dict(W_st0=W_st0, b_st0=b_st0, W_ts0=W_ts0, b_ts0=b_ts0,
                   W_st1=W_st1, b_st1=b_st1, W_ts1=W_ts1, b_ts1=b_ts1,
                   W_last=W_last, b_last=b_last)
    out = run(cfg, x, edge_index, is_reversed, weights)
    return out.astype(np.float32)


# revision 13
# speedup vs baseline: 1.2345x; 1.0953x over previous
"""Trainium2 Bass kernel v2 for nn_BiModel (2-layer bidirectional GCN).

Distribution over 8 NeuronCores, nodes sharded 6250/core.

Structure (vs v1):
- Each layer AllGathers its bf16 message table in two per-core half
  slices so the second collective overlaps with aggregation of the first
  half; gathers read the collective output directly (no DRAM copy).
- Node halves are block-aligned and padded: half 0 = local rows [0,3200),
  half 1 = [3200,6250) padded to 3072 rows.  Half tables hold
  8*3200=25600 / 8*3072=24576 rows (int16-gatherable).
- Edge chunks are capacity-padded per (branch, src-half, dst-block); both
  branches' one-hot matmuls pair into a single PSUM tile per dst block.
- Layer 2 aggregates prescaled h2 directly; W_last applied afterwards.
"""

import numpy as np

import concourse.bass as bass
import concourse.bacc as bacc
import concourse.mybir as mybir
import concourse.tile as tile
from concourse.bass_utils import run_bass_kernel_spmd
from concourse.masks import make_identity

import ml_dtypes

P = 128
F32 = mybir.dt.float32
BF16 = mybir.dt.bfloat16
I16 = mybir.dt.int16
I32 = mybir.dt.int32

FULL_CFG = dict(n=50000, e=800000, f_in=500, h=64, c_out=16, n_cores=8,
                cap_floor=5, blocks_per_group=5, hl=3200)


def cdiv(a, b):
    return (a + b - 1) // b


# ----------------------------------------------------------------------------
# host-side layout / preprocessing
# ----------------------------------------------------------------------------

class Layout2:
    """Compile-time layout shared by all cores (uniform SPMD program).
    Edge chunk capacity per (branch, src-half, dst-block) = max count over
    cores rounded up to 128 chunks, floored at cap_floor chunks."""

    def __init__(self, cfg, counts):
        # counts: [n_cores, 2, 2, nblk] (core, branch, src-half, dst-blk)
        self.cfg = cfg
        self.nloc = cfg["n"] // cfg["n_cores"]
        self.nblk = cdiv(self.nloc, P)
        hl = cfg["hl"]
        self.hpad = [hl, self.nblk * P - hl]
        self.htot = [hp * cfg["n_cores"] for hp in self.hpad]
        cap = counts.max(axis=0)                      # [2, 2, nblk]
        self.cap_chunks = np.maximum(cdiv(cap, P), cfg["cap_floor"])
        self.chunk_off = np.zeros((2, 2, self.nblk), np.int64)
        self.nchunks_bh = np.zeros((2, 2), np.int64)
        for b in range(2):
            for h in range(2):
                off = 0
                for blk in range(self.nblk):
                    self.chunk_off[b, h, blk] = off
                    off += self.cap_chunks[b, h, blk]
                self.nchunks_bh[b, h] = off
        bg = cfg["blocks_per_group"]
        self.groups = [list(range(g * bg, min((g + 1) * bg, self.nblk)))
                       for g in range(cdiv(self.nblk, bg))]

    def signature(self):
        return (tuple(self.cap_chunks.reshape(-1).tolist()),
                tuple(sorted(self.cfg.items())))


def _wrap_idx16(idx, n_pad):
    buf = np.zeros(n_pad, np.int16)
    buf[: len(idx)] = idx.astype(np.int16)
    w = buf.reshape(n_pad // 16, 16).T            # [16, n/16]
    return np.ascontiguousarray(np.tile(w, (8, 1)))  # [128, n/16]


def host_prep(cfg, x, edge_index, is_reversed):
    n, f_in = cfg["n"], cfg["f_in"]
    n_cores = cfg["n_cores"]
    nloc = n // n_cores
    nblk = cdiv(nloc, P)
    f_pad = cdiv(f_in, P) * P
    kch = f_pad // P
    hl = cfg["hl"]
    hpad = [hl, nblk * P - hl]

    src = np.asarray(edge_index[0], np.int64)
    dst = np.asarray(edge_index[1], np.int64)
    rev = np.asarray(is_reversed).astype(bool)

    core = dst // nloc
    dl = dst % nloc
    blk = dl // P
    branch = rev.astype(np.int64)
    cs = src // nloc
    rs = src % nloc
    hf = (rs >= hl).astype(np.int64)               # src half
    tblidx = cs * np.where(hf == 0, hpad[0], hpad[1]) + (rs - hf * hl)

    key = (((core * 2 + branch) * 2 + hf) * nblk) + blk
    order = np.argsort(key, kind="stable")
    counts = np.bincount(key[order], minlength=n_cores * 2 * 2 * nblk)
    counts = counts.reshape(n_cores, 2, 2, nblk)
    lay = Layout2(cfg, counts)

    deg = np.zeros((2, n), np.float32)
    np.add.at(deg[0], dst[~rev], 1.0)
    np.add.at(deg[1], dst[rev], 1.0)

    # node-feature transpose, bf16, padded
    xT = np.zeros((f_pad, n), ml_dtypes.bfloat16)
    xT[:f_in] = np.asarray(x, np.float32).T

    tbl_s = tblidx[order]
    dl_s = dl[order]
    gs = np.concatenate([[0], np.cumsum(counts.reshape(-1))])[:-1]
    gs = gs.reshape(n_cores, 2, 2, nblk)

    nblk_pad = nblk * P
    in_maps = []
    for c in range(n_cores):
        xc = xT[:, c * nloc:(c + 1) * nloc].reshape(kch, P, nloc)
        m = {"xT": np.ascontiguousarray(
            xc.transpose(1, 0, 2).reshape(P, kch * nloc))}
        degs = np.ones((P, 2 * nblk), np.float32)
        for b in range(2):
            dloc = np.ones(nblk_pad, np.float32)
            dloc[:nloc] = deg[b, c * nloc:(c + 1) * nloc]
            degs[:, b * nblk:(b + 1) * nblk] = dloc.reshape(nblk, P).T
        m["degs"] = degs
        for b in range(2):
            for h in range(2):
                nch = max(int(lay.nchunks_bh[b, h]), 1)
                tot = nch * P
                idx_stream = np.zeros(tot, np.int16)
                dstv = np.full(tot, -1.0, np.float32)
                for blk_ in range(nblk):
                    cnt = int(counts[c, b, h, blk_])
                    s0 = int(gs[c, b, h, blk_])
                    co = int(lay.chunk_off[b, h, blk_]) * P
                    idx_stream[co:co + cnt] = tbl_s[s0:s0 + cnt]
                    dstv[co:co + cnt] = dl_s[s0:s0 + cnt] - blk_ * P
                m[f"idx_b{b}h{h}"] = _wrap_idx16(idx_stream, tot)
                m[f"dst_b{b}h{h}"] = np.ascontiguousarray(
                    dstv.reshape(nch, P).T)          # [128, nch]
        in_maps.append(m)
    return lay, in_maps


def host_prep_weights(cfg, W_st0, b_st0, W_ts0, b_ts0, W_st1, b_st1,
                      W_ts1, b_ts1, W_last, b_last):
    f_in, h, c_out = cfg["f_in"], cfg["h"], cfg["c_out"]
    f_pad = cdiv(f_in, P) * P
    W0 = np.zeros((f_pad, 2 * h), np.float32)
    W0[:f_in, :h] = W_st0
    W0[:f_in, h:] = W_ts0
    kch = f_pad // P
    W0 = np.ascontiguousarray(
        W0.reshape(kch, P, 2 * h).transpose(1, 0, 2).reshape(P, kch * 2 * h))
    W1 = np.concatenate([W_st1, W_ts1], axis=1).astype(np.float32)
    WL = np.zeros((2 * h, 128), np.float32)
    WL[:, :c_out] = W_last
    bias01 = np.stack([np.concatenate([b_st0, b_ts0]),
                       np.concatenate([b_st1, b_ts1])], axis=1).astype(np.float32)
    return dict(W0=W0.astype(ml_dtypes.bfloat16),
                W1=W1.astype(ml_dtypes.bfloat16),
                WL=WL.astype(ml_dtypes.bfloat16), WLf=WL, bias01=bias01,
                b_last=np.asarray(b_last, np.float32).reshape(c_out, 1))


# ----------------------------------------------------------------------------
# device program
# ----------------------------------------------------------------------------

def build_program(cfg, lay, repeat=1, layers=3, no_coll=False):
    n, f_in = cfg["n"], cfg["f_in"]
    h, c_out = cfg["h"], cfg["c_out"]
    n_cores = cfg["n_cores"]
    nloc = n // n_cores
    nblk = lay.nblk
    nblk_pad = nblk * P
    f_pad = cdiv(f_in, P) * P
    kch = f_pad // P
    h2 = 2 * h
    hpad = lay.hpad
    htot = lay.htot
    hblk = [hpad[0] // P, hpad[1] // P]           # local blocks per half
    core_ids = list(range(n_cores))

    nc = bacc.Bacc("TRN2", target_bir_lowering=False, debug=False,
                   num_devices=n_cores)

    xT_d = nc.declare_dram_parameter("xT", [P, kch * nloc], BF16, isOutput=False)
    degs_d = nc.declare_dram_parameter("degs", [P, 2 * nblk], F32, isOutput=False)
    W0_d = nc.declare_dram_parameter("W0", [P, kch * h2], BF16, isOutput=False)
    W1_d = nc.declare_dram_parameter("W1", [h2, h2], BF16, isOutput=False)
    WL_d = nc.declare_dram_parameter("WL", [h2, 128], BF16, isOutput=False)
    WLf_d = nc.declare_dram_parameter("WLf", [h2, 128], F32, isOutput=False)
    bias01_d = nc.declare_dram_parameter("bias01", [h2, 2], F32, isOutput=False)
    b_last_d = nc.declare_dram_parameter("b_last", [c_out, 1], F32, isOutput=False)
    idx_d, dst_d = {}, {}
    for b in range(2):
        for hf in range(2):
            ncw = max(int(lay.nchunks_bh[b, hf]), 1)
            idx_d[b, hf] = nc.declare_dram_parameter(
                f"idx_b{b}h{hf}", [P, ncw * 8], I16, isOutput=False)
            dst_d[b, hf] = nc.declare_dram_parameter(
                f"dst_b{b}h{hf}", [P, ncw], F32, isOutput=False)
    out_d = nc.declare_dram_parameter("out", [nloc, c_out], F32, isOutput=True)

    tbl_loc_h = [nc.dram_tensor(f"tblloc_h{i}", [hpad[i], h2], BF16)
                 for i in range(2)]
    tbl_half = [nc.dram_tensor(f"tbl_h{i}", [htot[i], h2], BF16,
                               addr_space="Shared") for i in range(2)]

    with tile.TileContext(nc) as tc:
        with (
            tc.tile_pool(name="persist", bufs=1) as pp,
            tc.tile_pool(name="init", bufs=1) as ip,
            tc.tile_pool(name="work", bufs=2) as wp,
            tc.tile_pool(name="xload", bufs=3) as xp,
            tc.tile_pool(name="msg", bufs=2) as mp,
            tc.tile_pool(name="mask", bufs=6) as kp,
            tc.tile_pool(name="psA", bufs=3, space="PSUM") as psA,
            tc.tile_pool(name="psN", bufs=2, space="PSUM") as psN,
            tc.tile_pool(name="psB", bufs=2, space="PSUM") as psB,
        ):
            # ---------- constants ----------
            iota_i = ip.tile([P, P], I32, tag="ioi")
            nc.gpsimd.iota(iota_i[:], pattern=[[1, P]], base=0,
                           channel_multiplier=0)
            iota_bf = pp.tile([P, P], BF16, tag="iobf")
            nc.vector.tensor_copy(iota_bf[:], iota_i[:])
            ident = pp.tile([P, P], F32, tag="ident")
            make_identity(nc, ident[:])
            ident_bf = pp.tile([P, P], BF16, tag="identbf")
            nc.vector.tensor_copy(ident_bf[:], ident[:])
            biasv = pp.tile([P, 2], F32, tag="biasv")
            nc.sync.dma_start(out=biasv[:], in_=bias01_d[:, :])
            biasL = pp.tile([c_out, 1], F32, tag="biasL")
            nc.sync.dma_start(out=biasL[:], in_=b_last_d[:, :])

            # ---------- local degrees -> dinv [128, 3*nblk] (st|ts|all) ----
            deg_sb = ip.tile([P, 2 * nblk], F32, tag="degsb")
            nc.sync.dma_start(out=deg_sb[:], in_=degs_d[:, :])
            dtmp = ip.tile([P, 3 * nblk], F32, tag="dtmp")
            nc.vector.tensor_tensor(out=dtmp[:, 2 * nblk:],
                                    in0=deg_sb[:, :nblk], in1=deg_sb[:, nblk:],
                                    op=mybir.AluOpType.add)
            nc.vector.tensor_copy(dtmp[:, :2 * nblk], deg_sb[:])
            nc.vector.tensor_scalar_add(dtmp[:], dtmp[:], 1.0)
            dsq = ip.tile([P, 3 * nblk], F32, tag="dsq")
            nc.scalar.sqrt(dsq[:], dtmp[:])
            dinv = pp.tile([P, 3 * nblk], F32, tag="dinv")
            nc.vector.reciprocal(dinv[:], dsq[:])


            # dinv broadcast rows [128, nblk_pad] (st rows 0:64, ts 64:128)
            # and [c_out, nblk_pad] of dinv_all
            dinvT = ip.tile([nblk, 3 * P], F32, tag="dinvT")
            for i in range(3):
                tps = psB.tile([nblk, P], F32, tag="pst")
                nc.tensor.transpose(tps[:], dinv[:, i * nblk:(i + 1) * nblk],
                                    ident[:])
                nc.scalar.copy(dinvT[:, i * P:(i + 1) * P], tps[:])
            dinv_flat_d = nc.dram_tensor("dinv_flat", [1, 3 * nblk_pad], F32)
            for i in range(3):
                nc.sync.dma_start(
                    out=dinv_flat_d[0:1, i * nblk_pad:(i + 1) * nblk_pad],
                    in_=dinvT[:, i * P:(i + 1) * P])
            ones_row = pp.tile([1, P], F32, tag="ones_row")
            nc.vector.memset(ones_row[:], 1.0)
            dinvb = pp.tile([P, nblk_pad], F32, tag="dinvb")
            dinvallb = pp.tile([c_out, nblk_pad], F32, tag="dinvallb")
            NTB = 512
            for t0 in range(0, nblk_pad, NTB):
                t1 = min(t0 + NTB, nblk_pad)
                dfs = ip.tile([1, 3 * NTB], F32, tag="dfs")
                for i in range(3):
                    nc.sync.dma_start(
                        out=dfs[0:1, i * NTB: i * NTB + t1 - t0],
                        in_=dinv_flat_d[0:1, i * nblk_pad + t0: i * nblk_pad + t1])
                bps = psB.tile([P, NTB], F32, tag="pst")
                nc.tensor.matmul(bps[0:h, :t1 - t0], lhsT=ones_row[0:1, 0:h],
                                 rhs=dfs[0:1, 0:t1 - t0],
                                 start=True, stop=True)
                nc.tensor.matmul(bps[h:h2, :t1 - t0], lhsT=ones_row[0:1, 0:h],
                                 rhs=dfs[0:1, NTB:NTB + t1 - t0],
                                 start=True, stop=True, tile_position=(0, h))
                nc.scalar.copy(dinvb[:, t0:t1], bps[:, :t1 - t0])
                bps2 = psB.tile([P, NTB], F32, tag="pst")
                nc.tensor.matmul(bps2[:c_out, :t1 - t0],
                                 lhsT=ones_row[0:1, 0:c_out],
                                 rhs=dfs[0:1, 2 * NTB:2 * NTB + t1 - t0],
                                 start=True, stop=True)
                nc.scalar.copy(dinvallb[:, t0:t1], bps2[:c_out, :t1 - t0])

            # ---------- weights ----------
            w0_sb = pp.tile([P, kch * h2], BF16, tag="w0")
            nc.sync.dma_start(out=w0_sb[:], in_=W0_d[:, :])
            w1_sb = pp.tile([P, h2], BF16, tag="w1")
            nc.sync.dma_start(out=w1_sb[:], in_=W1_d[:, :])
            wl_sb = pp.tile([P, 128], BF16, tag="wl")
            nc.sync.dma_start(out=wl_sb[:], in_=WL_d[:, :])
            wlf_sb = pp.tile([P, 128], F32, tag="wlf")
            nc.sync.dma_start(out=wlf_sb[:], in_=WLf_d[:, :])

            # ---------- edge chunk idx / dstloc resident in SBUF ----------
            idx_sb, dst_sb = {}, {}
            for b_ in range(2):
                for hf_ in range(2):
                    ncw = max(int(lay.nchunks_bh[b_, hf_]), 1)
                    ti = pp.tile([P, ncw * 8], I16, tag=f"idxsb{b_}{hf_}")
                    nc.sync.dma_start(out=ti[:], in_=idx_d[b_, hf_][:, :])
                    idx_sb[b_, hf_] = ti
                    td = pp.tile([P, ncw], F32, tag=f"dstsb{b_}{hf_}")
                    nc.sync.dma_start(out=td[:], in_=dst_d[b_, hf_][:, :])
                    dst_sb[b_, hf_] = td

            # ---------- state ----------
            hT = pp.tile([P, nblk_pad], BF16, tag="hT")
            h2T = pp.tile([P, nblk_pad], BF16, tag="h2T")
            xwT = pp.tile([P, nblk_pad], F32, tag="xwT")
            aggT = pp.tile([P, nblk_pad], F32, tag="aggT")
            xwTL = xwT[0:c_out, :]
            outTL = aggT[0:c_out, :]

            # ---------------------------------------------------------------
            def build_xw_featmajor(src_getter, src_kch, w_ap_of_k, rows, dst):
                NT = 512
                for t0 in range(0, nloc, NT):
                    t1 = min(t0 + NT, nloc)
                    ps = psB.tile([P, NT], F32, tag="pst")
                    for k in range(src_kch):
                        nc.tensor.matmul(
                            ps[:rows, :t1 - t0],
                            lhsT=w_ap_of_k(k)[:, :rows],
                            rhs=src_getter(k, t0, t1),
                            start=(k == 0), stop=(k == src_kch - 1))
                    nc.scalar.copy(dst[:rows, t0:t1], ps[:rows, :t1 - t0])

            def build_table_rows(src_ap, w_ap, tbl_dst, blk_lo, blk_hi,
                                 row_base, prescale):
                """node-major prescaled bf16 table rows from feature-major
                SBUF source (single k chunk); all blocks full (padded)."""
                for blk in range(blk_lo, blk_hi):
                    nb0 = blk * P
                    ps = psN.tile([P, h2], F32, tag="psnm")
                    nc.tensor.matmul(ps[:, :], lhsT=src_ap[:, nb0:nb0 + P],
                                     rhs=w_ap, start=True, stop=True)
                    tt = wp.tile([P, h2], BF16, tag="tblt")
                    for (c0, c1, dcol) in prescale:
                        nc.vector.tensor_scalar_mul(
                            tt[:, c0:c1], ps[:, c0:c1],
                            dinv[:, dcol * nblk + blk: dcol * nblk + blk + 1])
                    nc.sync.dma_start(
                        out=tbl_dst[nb0 - row_base:nb0 - row_base + P, 0:h2],
                        in_=tt[:, :])

            # ---------------------------------------------------------------
            def aggregate(tables, full_rows):
                """Gather + one-hot-matmul segment sums over the 4 edge
                streams (branch x src-half).  full_rows=False: branch b uses
                lhs cols/psum rows [b*h,(b+1)*h).  full_rows=True (layer 2):
                full 128-wide lhs, both branches accumulate into all rows."""
                nmask = [0]

                def build_mask(dstcol_ap):
                    mk = kp.tile([P, P], BF16, tag="mask")
                    eng = nc.vector if nmask[0] % 3 else nc.gpsimd
                    nmask[0] += 1
                    eng.tensor_scalar(
                        out=mk[:], in0=iota_bf[:], scalar1=dstcol_ap,
                        scalar2=None, op0=mybir.AluOpType.is_equal)
                    return mk

                for hf in range(2):
                    for blocks in lay.groups:
                        bufs, dls = {}, {}
                        for b in range(2):
                            ch0 = int(lay.chunk_off[b, hf, blocks[0]])
                            ch1 = int(lay.chunk_off[b, hf, blocks[-1]]
                                      + lay.cap_chunks[b, hf, blocks[-1]])
                            nch = ch1 - ch0
                            it = idx_sb[b, hf][:, ch0 * 8: ch1 * 8]
                            dt = dst_sb[b, hf][:, ch0:ch1]
                            buf = mp.tile([P, nch, h2], BF16, tag=f"msg{b}")
                            nidx = nch * P
                            nc.gpsimd.dma_gather(
                                out_ap=buf[:], in_ap=tables[hf][:, :],
                                idxs_ap=it, num_idxs=nidx,
                                num_idxs_reg=nidx, elem_size=h2,
                                single_packet=(nidx <= 1024))
                            bufs[b] = (buf, ch0)
                            dls[b] = (dt, ch0)
                        for blk in blocks:
                            nb = slice(blk * P, (blk + 1) * P)
                            ps = psA.tile([P, P], F32, tag="agg")
                            for b in range(2):
                                buf, ch0 = bufs[b]
                                dt, dc0 = dls[b]
                                ncap = int(lay.cap_chunks[b, hf, blk])
                                co = int(lay.chunk_off[b, hf, blk])
                                for j in range(ncap):
                                    mk = build_mask(
                                        dt[:, co - dc0 + j: co - dc0 + j + 1])
                                    if full_rows:
                                        lh = buf[:, co - ch0 + j, 0:h2]
                                        o = ps[:, :]
                                        tpos = None
                                        st = (b == 0 and j == 0)
                                        sp = (b == 1 and j == ncap - 1)
                                    else:
                                        lh = buf[:, co - ch0 + j,
                                                 b * h:(b + 1) * h]
                                        o = ps[b * h:(b + 1) * h, :]
                                        tpos = (0, b * h)
                                        st = (j == 0)
                                        sp = (j == ncap - 1)
                                    nc.tensor.matmul(o, lhsT=lh, rhs=mk[:],
                                                     start=st, stop=sp,
                                                     tile_position=tpos)
                            if hf == 0:
                                nc.scalar.copy(aggT[:, nb], ps[:, :])
                            else:
                                nc.vector.tensor_add(out=aggT[:, nb],
                                                     in0=aggT[:, nb],
                                                     in1=ps[:, :])

            # ---------------------------------------------------------------
            def post01(layer, out_tile):
                nc.vector.tensor_tensor(out=xwT[:, :], in0=xwT[:, :],
                                        in1=dinvb[:, :],
                                        op=mybir.AluOpType.mult)
                nc.vector.tensor_tensor(out=aggT[:, :], in0=aggT[:, :],
                                        in1=xwT[:, :],
                                        op=mybir.AluOpType.add)
                nc.vector.tensor_tensor(out=aggT[:, :], in0=aggT[:, :],
                                        in1=dinvb[:, :],
                                        op=mybir.AluOpType.mult)
                nc.scalar.activation(out_tile[:, :], aggT[:, :],
                                     mybir.ActivationFunctionType.Relu,
                                     bias=biasv[:, layer:layer + 1])

            # ---------------------------------------------------------------
            def _phases():
                if nblk_pad > nloc:
                    nc.vector.memset(xwT[:, nloc:], 0.0)
                    nc.vector.memset(hT[:, nloc:], 0.0)
                    nc.vector.memset(h2T[:, nloc:], 0.0)

                # =================== layer 0 ===================
                with nc.named_scope("L0_tables"):
                    # zero padded tail rows of the half-1 local table once
                    zpad = hpad[1] * P - (nloc - hpad[0] * 1)
                    padrows = hpad[0] + hpad[1] - nloc      # 22
                    if padrows > 0:
                        zt = wp.tile([padrows, h2], BF16, tag="zpad")
                        nc.vector.memset(zt[:], 0.0)
                        nc.sync.dma_start(
                            out=tbl_loc_h[1][hpad[1] - padrows:hpad[1], :],
                            in_=zt[:])
                    NT = 512
                    for t0 in range(0, nloc, NT):
                        t1 = min(t0 + NT, nloc)
                        xts = []
                        for k in range(kch):
                            t = xp.tile([P, NT], BF16, tag=f"xb{k}")
                            nc.sync.dma_start(
                                out=t[:, :t1 - t0],
                                in_=xT_d[:, k * nloc + t0: k * nloc + t1])
                            xts.append(t)
                        # feature-major xw for the self-loop term
                        ps = psB.tile([P, NT], F32, tag="pst")
                        for k in range(kch):
                            nc.tensor.matmul(
                                ps[:h2, :t1 - t0],
                                lhsT=w0_sb[:, k * h2:(k + 1) * h2],
                                rhs=xts[k][:, :t1 - t0],
                                start=(k == 0), stop=(k == kch - 1))
                        nc.scalar.copy(xwT[:h2, t0:t1], ps[:h2, :t1 - t0])
                        # node-major prescaled table rows
                        for blk in range(t0 // P, cdiv(t1, P)):
                            nb0 = blk * P
                            nn = min(P, nloc - nb0)
                            psn = psN.tile([P, h2], F32, tag="psnm")
                            for k in range(kch):
                                nc.tensor.matmul(
                                    psn[:nn, :],
                                    lhsT=xts[k][:, nb0 - t0:nb0 - t0 + nn],
                                    rhs=w0_sb[:, k * h2:(k + 1) * h2],
                                    start=(k == 0), stop=(k == kch - 1))
                            tt = wp.tile([P, h2], BF16, tag="tblt")
                            for (cc0, cc1, dcol) in ((0, h, 0), (h, h2, 1)):
                                nc.vector.tensor_scalar_mul(
                                    tt[:nn, cc0:cc1], psn[:nn, cc0:cc1],
                                    dinv[:nn, dcol * nblk + blk:
                                         dcol * nblk + blk + 1])
                            hf = 0 if blk < hblk[0] else 1
                            r0 = nb0 - (hpad[0] if hf else 0)
                            nc.sync.dma_start(
                                out=tbl_loc_h[hf][r0:r0 + nn, :],
                                in_=tt[:nn, :])
                    if not no_coll:
                        for hf in range(2):
                            nc.gpsimd.collective_compute(
                                "AllGather", mybir.AluOpType.bypass,
                                replica_groups=[core_ids],
                                ins=[tbl_loc_h[hf][:]], outs=[tbl_half[hf][:]])
                with nc.named_scope("L0_agg"):
                    aggregate(tbl_half, False)
                with nc.named_scope("L0_post"):
                    post01(0, hT)
                if layers <= 1:
                    nc.sync.dma_start(out=out_d[0:P, :], in_=xwT[0:P, 0:c_out])
                    return

                # =================== layer 1 ===================
                with nc.named_scope("L1_tables"):
                    for hf in range(2):
                        build_table_rows(hT, w1_sb[:], tbl_loc_h[hf],
                                         0 if hf == 0 else hblk[0],
                                         hblk[0] if hf == 0 else nblk,
                                         0 if hf == 0 else hpad[0],
                                         ((0, h, 0), (h, h2, 1)))
                        if not no_coll:
                            nc.gpsimd.collective_compute(
                                "AllGather", mybir.AluOpType.bypass,
                                replica_groups=[core_ids],
                                ins=[tbl_loc_h[hf][:]], outs=[tbl_half[hf][:]])
                    build_xw_featmajor(lambda k, a, bb: hT[:, a:bb], 1,
                                       lambda k: w1_sb[:], h2, xwT)
                with nc.named_scope("L1_agg"):
                    aggregate(tbl_half, False)
                with nc.named_scope("L1_post"):
                    post01(1, h2T)
                if layers <= 2:
                    nc.sync.dma_start(out=out_d[0:P, :], in_=xwT[0:P, 0:c_out])
                    return

                # =================== layer 2 ===================
                with nc.named_scope("L2_tables"):
                    for hf in range(2):
                        build_table_rows(h2T, ident_bf[:], tbl_loc_h[hf],
                                         0 if hf == 0 else hblk[0],
                                         hblk[0] if hf == 0 else nblk,
                                         0 if hf == 0 else hpad[0],
                                         ((0, h2, 2),))
                        if not no_coll:
                            nc.gpsimd.collective_compute(
                                "AllGather", mybir.AluOpType.bypass,
                                replica_groups=[core_ids],
                                ins=[tbl_loc_h[hf][:]], outs=[tbl_half[hf][:]])
                    build_xw_featmajor(lambda k, a, bb: h2T[:, a:bb], 1,
                                       lambda k: wl_sb[:], c_out, xwT)
                with nc.named_scope("L2_agg"):
                    aggregate(tbl_half, True)

                # out16 = (WL^T aggT)*dinvall + xwTL*dinvall^2 + b_last
                with nc.named_scope("L2_post"):
                    NT = 512
                    for t0 in range(0, nblk_pad, NT):
                        t1 = min(t0 + NT, nblk_pad)
                        ps = psB.tile([P, NT], F32, tag="pst")
                        nc.tensor.matmul(ps[:c_out, :t1 - t0],
                                         lhsT=wlf_sb[:, :c_out],
                                         rhs=aggT[:, t0:t1],
                                         start=True, stop=True)
                        nc.scalar.copy(outTL[:, t0:t1], ps[:c_out, :t1 - t0])
                    nc.vector.tensor_tensor(out=xwTL[:, :], in0=xwTL[:, :],
                                            in1=dinvallb[:, :],
                                            op=mybir.AluOpType.mult)
                    nc.vector.tensor_tensor(out=outTL[:, :], in0=outTL[:, :],
                                            in1=xwTL[:, :],
                                            op=mybir.AluOpType.add)
                    nc.vector.tensor_tensor(out=outTL[:, :], in0=outTL[:, :],
                                            in1=dinvallb[:, :],
                                            op=mybir.AluOpType.mult)
                    nc.scalar.activation(outTL[:, :], outTL[:, :],
                                         mybir.ActivationFunctionType.Identity,
                                         bias=biasL[:, 0:1])

                with nc.named_scope("softmax"):
                    for blk in range(nblk):
                        nb0 = blk * P
                        nb1 = min(nb0 + P, nloc)
                        nn = nb1 - nb0
                        if nn <= 0:
                            continue
                        tp = psB.tile([P, c_out], F32, tag="pst")
                        nc.tensor.transpose(tp[:], outTL[:, nb0:nb0 + P],
                                            ident[:c_out, :c_out])
                        negmax = wp.tile([P, 1], F32, tag="negmax")
                        nc.vector.tensor_reduce(negmax[:], tp[:],
                                                axis=mybir.AxisListType.X,
                                                op=mybir.AluOpType.max,
                                                negate=True)
                        ex = wp.tile([P, c_out], F32, tag="ex")
                        nc.scalar.activation(ex[:], tp[:],
                                             mybir.ActivationFunctionType.Exp,
                                             bias=negmax[:, 0:1])
                        sume = wp.tile([P, 1], F32, tag="sume")
                        nc.vector.tensor_reduce(sume[:], ex[:],
                                                axis=mybir.AxisListType.X,
                                                op=mybir.AluOpType.add)
                        lse = wp.tile([P, 1], F32, tag="lse")
                        nc.scalar.activation(lse[:], sume[:],
                                             mybir.ActivationFunctionType.Ln)
                        fin = wp.tile([P, c_out], F32, tag="fin")
                        nc.vector.tensor_scalar(
                            out=fin[:], in0=tp[:], scalar1=negmax[:, 0:1],
                            scalar2=lse[:, 0:1], op0=mybir.AluOpType.add,
                            op1=mybir.AluOpType.subtract)
                        nc.sync.dma_start(out=out_d[nb0:nb1, :], in_=fin[:nn, :])

            for _rep in range(repeat):
                _phases()

    nc.compile()
    return nc


# ----------------------------------------------------------------------------
# driver
# ----------------------------------------------------------------------------

_CACHE = {}
_RUNNER = {}


def _build_runner(nc, n_cores):
    import jax
    from jax.sharding import Mesh, PartitionSpec
    from jax.experimental.shard_map import shard_map
    import concourse.mybir as mybir_
    from concourse import bass2jax
    from concourse.bass2jax import _bass_exec_p, partition_id_tensor

    bass2jax.install_neuronx_cc_hook()
    partition_name = (nc.partition_id_tensor.name
                      if nc.partition_id_tensor else None)
    in_names, out_names, out_avals, zero_outs = [], [], [], []
    for alloc in nc.m.functions[0].allocations:
        if not isinstance(alloc, mybir_.MemoryLocationSet):
            continue
        name = alloc.memorylocations[0].name
        if alloc.kind == "ExternalInput":
            if name != partition_name:
                in_names.append(name)
        elif alloc.kind == "ExternalOutput":
            out_names.append(name)
            shape = tuple(alloc.tensor_shape)
            dtype = mybir_.dt.np(alloc.dtype)
            out_avals.append(jax.core.ShapedArray(shape, dtype))
            zero_outs.append(np.zeros(shape, dtype))
    n_params = len(in_names)
    all_names = in_names + out_names
    if partition_name is not None:
        all_names.append(partition_name)

    def _body(*args):
        operands = list(args)
        if partition_name is not None:
            operands.append(partition_id_tensor())
        return tuple(_bass_exec_p.bind(
            *operands, out_avals=tuple(out_avals), in_names=tuple(all_names),
            out_names=tuple(out_names), lowering_input_output_aliases=(),
            sim_require_finite=True, sim_require_nnan=True, nc=nc))

    devices = jax.devices()[:n_cores]
    mesh = Mesh(np.asarray(devices), ("core",))
    n_out = len(out_names)
    fn = jax.jit(shard_map(_body, mesh=mesh,
                           in_specs=(PartitionSpec("core"),) * (n_params + n_out),
                           out_specs=(PartitionSpec("core"),) * n_out,
                           check_rep=False), keep_unused=True)
    return fn, in_names, out_names, out_avals, zero_outs, mesh


def _run_persistent(nc, in_maps, n_cores, key):
    import jax
    if key not in _RUNNER:
        fn, in_names, out_names, out_avals, zero_outs, mesh = \
            _build_runner(nc, n_cores)
        _RUNNER[key] = dict(fn=fn, in_names=in_names, out_names=out_names,
                            out_avals=out_avals, zero_outs=zero_outs,
                            mesh=mesh, dev_args=None)
    R = _RUNNER[key]
    concat_in = [np.concatenate([np.asarray(in_maps[c][nm])
                                 for c in range(n_cores)], axis=0)
                 for nm in R["in_names"]]
    concat_zero = [np.zeros((n_cores * z.shape[0], *z.shape[1:]), z.dtype)
                   for z in R["zero_outs"]]
    args = [jax.device_put(a) for a in concat_in + concat_zero]
    R["dev_args"] = args
    outs = R["fn"](*args)
    outs = [np.asarray(o) for o in outs]
    return {nm: outs[i].reshape(n_cores, *R["out_avals"][i].shape)
            for i, nm in enumerate(R["out_names"])}


def run(cfg, x, edge_index, is_reversed, weights, use_sim=False, repeat=1,
        layers=3, no_coll=False):
    lay, in_maps = host_prep(cfg, x, edge_index, is_reversed)
    wmap = host_prep_weights(cfg, **weights)
    for m in in_maps:
        m.update(wmap)

    sig = (lay.signature(), repeat, layers, no_coll)
    if sig in _CACHE:
        nc = _CACHE[sig]
    else:
        nc = build_program(cfg, lay, repeat=repeat, layers=layers,
                           no_coll=no_coll)
        _CACHE[sig] = nc

    n_cores = cfg["n_cores"]
    if use_sim:
        import concourse.bass_interp as bass_interp
        sim = bass_interp.MultiCoreSim(nc, n_cores, require_finite=False,
                                       require_nnan=False)
        for c in range(n_cores):
            for k, v in in_maps[c].items():
                sim.cores[c].tensor(k)[:] = v
        sim.simulate()
        outs = [np.array(sim.cores[c].tensor("out")) for c in range(n_cores)]
    else:
        key = str(sig)
        res = _run_persistent(nc, in_maps, n_cores, key)
        outs = list(res["out"])
    return np.concatenate(outs, axis=0)


def _marginal_sample(fn, args, iters):
    import jax, time as _t
    t0 = _t.time()
    o = fn(*args); jax.block_until_ready(o)
    base = _t.time() - t0
    t0 = _t.time()
    for _ in range(1 + iters):
        o = fn(*args)
    jax.block_until_ready(o)
    return (_t.time() - t0 - base) / iters * 1e9


def _marginal_ns(key, iters=6, reps=3):
    import jax
    R = _RUNNER[key]
    fn, args = R["fn"], R["dev_args"]
    o = fn(*args); jax.block_until_ready(o)
    return min(_marginal_sample(fn, args, iters) for _ in range(reps))


def time_device(inputs, iters=6, cfg=None, repeat_hi=4):
    """On-device execution time per kernel invocation: difference of
    in-program R-repeat vs 1-repeat marginal wall times cancels the
    fixed per-dispatch (axon RPC) overhead."""
    cfg = cfg or FULL_CFG
    weights = {k: np.asarray(inputs[k]) for k in
               ("W_st0", "b_st0", "W_ts0", "b_ts0", "W_st1", "b_st1",
                "W_ts1", "b_ts1", "W_last", "b_last")}
    keys = {}
    layers = int(__import__("os").environ.get("K2_LAYERS", "3"))
    no_coll = bool(int(__import__("os").environ.get("K2_NOCOLL", "0")))
    for rep in (1, repeat_hi):
        run(cfg, inputs["x"], inputs["edge_index"], inputs["is_reversed"],
            weights, repeat=rep, layers=layers, no_coll=no_coll)
        lay, _ = host_prep(cfg, inputs["x"], inputs["edge_index"],
                           inputs["is_reversed"])
        keys[rep] = str((lay.signature(), rep, layers, no_coll))
    import jax
    R1, RH = _RUNNER[keys[1]], _RUNNER[keys[repeat_hi]]
    for R in (R1, RH):
        o = R["fn"](*R["dev_args"]); jax.block_until_ready(o)
    m1s, mRs = [], []
    for _ in range(5):
        m1s.append(_marginal_sample(R1["fn"], R1["dev_args"], max(iters, 8)))
        mRs.append(_marginal_sample(RH["fn"], RH["dev_args"], max(iters, 8)))
    return (min(mRs) - min(m1s)) / (repeat_hi - 1)


def kernel(x, edge_index, is_reversed, W_st0, b_st0, W_ts0, b_ts0,
           W_st1, b_st1, W_ts1, b_ts1, W_last, b_last):
    cfg = FULL_CFG
    weights = dict(W_st0=W_st0, b_st0=b_st0, W_ts0=W_ts0, b_ts0=b_ts0,
                   W_st1=W_st1, b_st1=b_st1, W_ts1=W_ts1, b_ts1=b_ts1,
                   W_last=W_last, b_last=b_last)
    out = run(cfg, x, edge_index, is_reversed, weights)
    return out.astype(np.float32)


# revision 14
# speedup vs baseline: 1.4725x; 1.1928x over previous
"""Trainium2 Bass kernel v2 for nn_BiModel (2-layer bidirectional GCN).

Distribution over 8 NeuronCores, nodes sharded 6250/core.

Structure (vs v1):
- Each layer AllGathers its bf16 message table in two per-core half
  slices so the second collective overlaps with aggregation of the first
  half; gathers read the collective output directly (no DRAM copy).
- Node halves are block-aligned and padded: half 0 = local rows [0,3200),
  half 1 = [3200,6250) padded to 3072 rows.  Half tables hold
  8*3200=25600 / 8*3072=24576 rows (int16-gatherable).
- Edge chunks are capacity-padded per (branch, src-half, dst-block); both
  branches' one-hot matmuls pair into a single PSUM tile per dst block.
- Layer 2 aggregates prescaled h2 directly; W_last applied afterwards.
"""

import numpy as np

import concourse.bass as bass
import concourse.bacc as bacc
import concourse.mybir as mybir
import concourse.tile as tile
from concourse.bass_utils import run_bass_kernel_spmd
from concourse.masks import make_identity

import ml_dtypes

P = 128
F32 = mybir.dt.float32
BF16 = mybir.dt.bfloat16
I16 = mybir.dt.int16
I32 = mybir.dt.int32

FULL_CFG = dict(n=50000, e=800000, f_in=500, h=64, c_out=16, n_cores=8,
                cap_floor=5, blocks_per_group=5, hl=3200, mask_mod=0)


def cdiv(a, b):
    return (a + b - 1) // b


# ----------------------------------------------------------------------------
# host-side layout / preprocessing
# ----------------------------------------------------------------------------

class Layout2:
    """Compile-time layout shared by all cores (uniform SPMD program).
    Edge chunk capacity per (branch, src-half, dst-block) = max count over
    cores rounded up to 128 chunks, floored at cap_floor chunks."""

    def __init__(self, cfg, counts):
        # counts: [n_cores, 2, 2, nblk] (core, branch, src-half, dst-blk)
        self.cfg = cfg
        self.nloc = cfg["n"] // cfg["n_cores"]
        self.nblk = cdiv(self.nloc, P)
        hl = cfg["hl"]
        self.hpad = [hl, self.nblk * P - hl]
        self.htot = [hp * cfg["n_cores"] for hp in self.hpad]
        cap = counts.max(axis=0)                      # [2, 2, nblk]
        self.cap_chunks = np.maximum(cdiv(cap, P), cfg["cap_floor"])
        self.chunk_off = np.zeros((2, 2, self.nblk), np.int64)
        self.nchunks_bh = np.zeros((2, 2), np.int64)
        for b in range(2):
            for h in range(2):
                off = 0
                for blk in range(self.nblk):
                    self.chunk_off[b, h, blk] = off
                    off += self.cap_chunks[b, h, blk]
                self.nchunks_bh[b, h] = off
        bg = cfg["blocks_per_group"]
        self.groups = [list(range(g * bg, min((g + 1) * bg, self.nblk)))
                       for g in range(cdiv(self.nblk, bg))]

    def signature(self):
        return (tuple(self.cap_chunks.reshape(-1).tolist()),
                tuple(sorted(self.cfg.items())))


def _wrap_idx16(idx, n_pad):
    buf = np.zeros(n_pad, np.int16)
    buf[: len(idx)] = idx.astype(np.int16)
    w = buf.reshape(n_pad // 16, 16).T            # [16, n/16]
    return np.ascontiguousarray(np.tile(w, (8, 1)))  # [128, n/16]


def host_prep(cfg, x, edge_index, is_reversed):
    n, f_in = cfg["n"], cfg["f_in"]
    n_cores = cfg["n_cores"]
    nloc = n // n_cores
    nblk = cdiv(nloc, P)
    f_pad = cdiv(f_in, P) * P
    kch = f_pad // P
    hl = cfg["hl"]
    hpad = [hl, nblk * P - hl]

    src = np.asarray(edge_index[0], np.int64)
    dst = np.asarray(edge_index[1], np.int64)
    rev = np.asarray(is_reversed).astype(bool)

    core = dst // nloc
    dl = dst % nloc
    blk = dl // P
    branch = rev.astype(np.int64)
    cs = src // nloc
    rs = src % nloc
    hf = (rs >= hl).astype(np.int64)               # src half
    tblidx = cs * np.where(hf == 0, hpad[0], hpad[1]) + (rs - hf * hl)

    key = (((core * 2 + branch) * 2 + hf) * nblk) + blk
    order = np.argsort(key, kind="stable")
    counts = np.bincount(key[order], minlength=n_cores * 2 * 2 * nblk)
    counts = counts.reshape(n_cores, 2, 2, nblk)
    lay = Layout2(cfg, counts)

    deg = np.zeros((2, n), np.float32)
    np.add.at(deg[0], dst[~rev], 1.0)
    np.add.at(deg[1], dst[rev], 1.0)

    # node-feature transpose, bf16, padded
    xT = np.zeros((f_pad, n), ml_dtypes.bfloat16)
    xT[:f_in] = np.asarray(x, np.float32).T

    tbl_s = tblidx[order]
    dl_s = dl[order]
    gs = np.concatenate([[0], np.cumsum(counts.reshape(-1))])[:-1]
    gs = gs.reshape(n_cores, 2, 2, nblk)

    nblk_pad = nblk * P
    in_maps = []
    for c in range(n_cores):
        xc = xT[:, c * nloc:(c + 1) * nloc].reshape(kch, P, nloc)
        m = {"xT": np.ascontiguousarray(
            xc.transpose(1, 0, 2).reshape(P, kch * nloc))}
        degs = np.ones((P, 2 * nblk), np.float32)
        for b in range(2):
            dloc = np.ones(nblk_pad, np.float32)
            dloc[:nloc] = deg[b, c * nloc:(c + 1) * nloc]
            degs[:, b * nblk:(b + 1) * nblk] = dloc.reshape(nblk, P).T
        m["degs"] = degs
        for b in range(2):
            for h in range(2):
                nch = max(int(lay.nchunks_bh[b, h]), 1)
                tot = nch * P
                idx_stream = np.zeros(tot, np.int16)
                dstv = np.full(tot, -1.0, np.float32)
                for blk_ in range(nblk):
                    cnt = int(counts[c, b, h, blk_])
                    s0 = int(gs[c, b, h, blk_])
                    co = int(lay.chunk_off[b, h, blk_]) * P
                    idx_stream[co:co + cnt] = tbl_s[s0:s0 + cnt]
                    dstv[co:co + cnt] = dl_s[s0:s0 + cnt] - blk_ * P
                m[f"idx_b{b}h{h}"] = _wrap_idx16(idx_stream, tot)
                m[f"dst_b{b}h{h}"] = np.ascontiguousarray(
                    dstv.reshape(nch, P).T)          # [128, nch]
        in_maps.append(m)
    return lay, in_maps


def host_prep_weights(cfg, W_st0, b_st0, W_ts0, b_ts0, W_st1, b_st1,
                      W_ts1, b_ts1, W_last, b_last):
    f_in, h, c_out = cfg["f_in"], cfg["h"], cfg["c_out"]
    f_pad = cdiv(f_in, P) * P
    W0 = np.zeros((f_pad, 2 * h), np.float32)
    W0[:f_in, :h] = W_st0
    W0[:f_in, h:] = W_ts0
    kch = f_pad // P
    W0 = np.ascontiguousarray(
        W0.reshape(kch, P, 2 * h).transpose(1, 0, 2).reshape(P, kch * 2 * h))
    W1 = np.concatenate([W_st1, W_ts1], axis=1).astype(np.float32)
    WL = np.zeros((2 * h, 128), np.float32)
    WL[:, :c_out] = W_last
    bias01 = np.stack([np.concatenate([b_st0, b_ts0]),
                       np.concatenate([b_st1, b_ts1])], axis=1).astype(np.float32)
    return dict(W0=W0.astype(ml_dtypes.bfloat16),
                W1=W1.astype(ml_dtypes.bfloat16),
                WL=WL.astype(ml_dtypes.bfloat16), WLf=WL, bias01=bias01,
                b_last=np.asarray(b_last, np.float32).reshape(c_out, 1))


# ----------------------------------------------------------------------------
# device program
# ----------------------------------------------------------------------------

def build_program(cfg, lay, repeat=1, layers=3, no_coll=False):
    n, f_in = cfg["n"], cfg["f_in"]
    h, c_out = cfg["h"], cfg["c_out"]
    n_cores = cfg["n_cores"]
    nloc = n // n_cores
    nblk = lay.nblk
    nblk_pad = nblk * P
    f_pad = cdiv(f_in, P) * P
    kch = f_pad // P
    h2 = 2 * h
    hpad = lay.hpad
    htot = lay.htot
    hblk = [hpad[0] // P, hpad[1] // P]           # local blocks per half
    core_ids = list(range(n_cores))

    nc = bacc.Bacc("TRN2", target_bir_lowering=False, debug=False,
                   num_devices=n_cores)

    xT_d = nc.declare_dram_parameter("xT", [P, kch * nloc], BF16, isOutput=False)
    degs_d = nc.declare_dram_parameter("degs", [P, 2 * nblk], F32, isOutput=False)
    W0_d = nc.declare_dram_parameter("W0", [P, kch * h2], BF16, isOutput=False)
    W1_d = nc.declare_dram_parameter("W1", [h2, h2], BF16, isOutput=False)
    WL_d = nc.declare_dram_parameter("WL", [h2, 128], BF16, isOutput=False)
    WLf_d = nc.declare_dram_parameter("WLf", [h2, 128], F32, isOutput=False)
    bias01_d = nc.declare_dram_parameter("bias01", [h2, 2], F32, isOutput=False)
    b_last_d = nc.declare_dram_parameter("b_last", [c_out, 1], F32, isOutput=False)
    idx_d, dst_d = {}, {}
    for b in range(2):
        for hf in range(2):
            ncw = max(int(lay.nchunks_bh[b, hf]), 1)
            idx_d[b, hf] = nc.declare_dram_parameter(
                f"idx_b{b}h{hf}", [P, ncw * 8], I16, isOutput=False)
            dst_d[b, hf] = nc.declare_dram_parameter(
                f"dst_b{b}h{hf}", [P, ncw], F32, isOutput=False)
    out_d = nc.declare_dram_parameter("out", [nloc, c_out], F32, isOutput=True)

    tbl_loc_h = [nc.dram_tensor(f"tblloc_h{i}", [hpad[i], h2], BF16)
                 for i in range(2)]
    tbl_half = [nc.dram_tensor(f"tbl_h{i}", [htot[i], h2], BF16,
                               addr_space="Shared") for i in range(2)]

    with tile.TileContext(nc) as tc:
        with (
            tc.tile_pool(name="persist", bufs=1) as pp,
            tc.tile_pool(name="init", bufs=1) as ip,
            tc.tile_pool(name="work", bufs=2) as wp,
            tc.tile_pool(name="xload", bufs=3) as xp,
            tc.tile_pool(name="msg", bufs=2) as mp,
            tc.tile_pool(name="mask", bufs=6) as kp,
            tc.tile_pool(name="psA", bufs=3, space="PSUM") as psA,
            tc.tile_pool(name="psN", bufs=2, space="PSUM") as psN,
            tc.tile_pool(name="psB", bufs=2, space="PSUM") as psB,
        ):
            # ---------- constants ----------
            iota_i = ip.tile([P, P], I32, tag="ioi")
            nc.gpsimd.iota(iota_i[:], pattern=[[1, P]], base=0,
                           channel_multiplier=0)
            iota_bf = pp.tile([P, P], BF16, tag="iobf")
            nc.vector.tensor_copy(iota_bf[:], iota_i[:])
            ident = pp.tile([P, P], F32, tag="ident")
            make_identity(nc, ident[:])
            ident_bf = pp.tile([P, P], BF16, tag="identbf")
            nc.vector.tensor_copy(ident_bf[:], ident[:])
            biasv = pp.tile([P, 2], F32, tag="biasv")
            nc.sync.dma_start(out=biasv[:], in_=bias01_d[:, :])
            biasL = pp.tile([c_out, 1], F32, tag="biasL")
            nc.sync.dma_start(out=biasL[:], in_=b_last_d[:, :])

            # ---------- local degrees -> dinv [128, 3*nblk] (st|ts|all) ----
            deg_sb = ip.tile([P, 2 * nblk], F32, tag="degsb")
            nc.sync.dma_start(out=deg_sb[:], in_=degs_d[:, :])
            dtmp = ip.tile([P, 3 * nblk], F32, tag="dtmp")
            nc.vector.tensor_tensor(out=dtmp[:, 2 * nblk:],
                                    in0=deg_sb[:, :nblk], in1=deg_sb[:, nblk:],
                                    op=mybir.AluOpType.add)
            nc.vector.tensor_copy(dtmp[:, :2 * nblk], deg_sb[:])
            nc.vector.tensor_scalar_add(dtmp[:], dtmp[:], 1.0)
            dsq = ip.tile([P, 3 * nblk], F32, tag="dsq")
            nc.scalar.sqrt(dsq[:], dtmp[:])
            dinv = pp.tile([P, 3 * nblk], F32, tag="dinv")
            nc.vector.reciprocal(dinv[:], dsq[:])


            # dinv broadcast rows [128, nblk_pad] (st rows 0:64, ts 64:128)
            # and [c_out, nblk_pad] of dinv_all
            dinvT = ip.tile([nblk, 3 * P], F32, tag="dinvT")
            for i in range(3):
                tps = psB.tile([nblk, P], F32, tag="pst")
                nc.tensor.transpose(tps[:], dinv[:, i * nblk:(i + 1) * nblk],
                                    ident[:])
                nc.scalar.copy(dinvT[:, i * P:(i + 1) * P], tps[:])
            dinv_flat_d = nc.dram_tensor("dinv_flat", [1, 3 * nblk_pad], F32)
            for i in range(3):
                nc.sync.dma_start(
                    out=dinv_flat_d[0:1, i * nblk_pad:(i + 1) * nblk_pad],
                    in_=dinvT[:, i * P:(i + 1) * P])
            ones_row = pp.tile([1, P], F32, tag="ones_row")
            nc.vector.memset(ones_row[:], 1.0)
            dinvb = pp.tile([P, nblk_pad], F32, tag="dinvb")
            dinvallb = pp.tile([c_out, nblk_pad], F32, tag="dinvallb")
            NTB = 512
            for t0 in range(0, nblk_pad, NTB):
                t1 = min(t0 + NTB, nblk_pad)
                dfs = ip.tile([1, 3 * NTB], F32, tag="dfs")
                for i in range(3):
                    nc.sync.dma_start(
                        out=dfs[0:1, i * NTB: i * NTB + t1 - t0],
                        in_=dinv_flat_d[0:1, i * nblk_pad + t0: i * nblk_pad + t1])
                bps = psB.tile([P, NTB], F32, tag="pst")
                nc.tensor.matmul(bps[0:h, :t1 - t0], lhsT=ones_row[0:1, 0:h],
                                 rhs=dfs[0:1, 0:t1 - t0],
                                 start=True, stop=True)
                nc.tensor.matmul(bps[h:h2, :t1 - t0], lhsT=ones_row[0:1, 0:h],
                                 rhs=dfs[0:1, NTB:NTB + t1 - t0],
                                 start=True, stop=True, tile_position=(0, h))
                nc.scalar.copy(dinvb[:, t0:t1], bps[:, :t1 - t0])
                bps2 = psB.tile([P, NTB], F32, tag="pst")
                nc.tensor.matmul(bps2[:c_out, :t1 - t0],
                                 lhsT=ones_row[0:1, 0:c_out],
                                 rhs=dfs[0:1, 2 * NTB:2 * NTB + t1 - t0],
                                 start=True, stop=True)
                nc.scalar.copy(dinvallb[:, t0:t1], bps2[:c_out, :t1 - t0])

            # ---------- weights ----------
            w0_sb = pp.tile([P, kch * h2], BF16, tag="w0")
            nc.sync.dma_start(out=w0_sb[:], in_=W0_d[:, :])
            w1_sb = pp.tile([P, h2], BF16, tag="w1")
            nc.sync.dma_start(out=w1_sb[:], in_=W1_d[:, :])
            wl_sb = pp.tile([P, 128], BF16, tag="wl")
            nc.sync.dma_start(out=wl_sb[:], in_=WL_d[:, :])
            wlf_sb = pp.tile([P, 128], F32, tag="wlf")
            nc.sync.dma_start(out=wlf_sb[:], in_=WLf_d[:, :])

            # ---------- edge chunk idx / dstloc resident in SBUF ----------
            idx_sb, dst_sb = {}, {}
            for b_ in range(2):
                for hf_ in range(2):
                    ncw = max(int(lay.nchunks_bh[b_, hf_]), 1)
                    ti = pp.tile([P, ncw * 8], I16, tag=f"idxsb{b_}{hf_}")
                    nc.sync.dma_start(out=ti[:], in_=idx_d[b_, hf_][:, :])
                    idx_sb[b_, hf_] = ti
                    td = pp.tile([P, ncw], F32, tag=f"dstsb{b_}{hf_}")
                    nc.sync.dma_start(out=td[:], in_=dst_d[b_, hf_][:, :])
                    dst_sb[b_, hf_] = td

            # ---------- state ----------
            hT = pp.tile([P, nblk_pad], BF16, tag="hT")
            h2T = pp.tile([P, nblk_pad], BF16, tag="h2T")
            xwT = pp.tile([P, nblk_pad], F32, tag="xwT")
            aggT = pp.tile([P, nblk_pad], F32, tag="aggT")
            xwTL = xwT[0:c_out, :]
            outTL = aggT[0:c_out, :]

            # ---------------------------------------------------------------
            def build_xw_featmajor(src_getter, src_kch, w_ap_of_k, rows, dst):
                NT = 512
                for t0 in range(0, nloc, NT):
                    t1 = min(t0 + NT, nloc)
                    ps = psB.tile([P, NT], F32, tag="pst")
                    for k in range(src_kch):
                        nc.tensor.matmul(
                            ps[:rows, :t1 - t0],
                            lhsT=w_ap_of_k(k)[:, :rows],
                            rhs=src_getter(k, t0, t1),
                            start=(k == 0), stop=(k == src_kch - 1))
                    nc.scalar.copy(dst[:rows, t0:t1], ps[:rows, :t1 - t0])

            def build_table_rows(src_ap, w_ap, tbl_dst, blk_lo, blk_hi,
                                 row_base, prescale):
                """node-major prescaled bf16 table rows from feature-major
                SBUF source (single k chunk); all blocks full (padded)."""
                for blk in range(blk_lo, blk_hi):
                    nb0 = blk * P
                    ps = psN.tile([P, h2], F32, tag="psnm")
                    nc.tensor.matmul(ps[:, :], lhsT=src_ap[:, nb0:nb0 + P],
                                     rhs=w_ap, start=True, stop=True)
                    tt = wp.tile([P, h2], BF16, tag="tblt")
                    for (c0, c1, dcol) in prescale:
                        nc.vector.tensor_scalar_mul(
                            tt[:, c0:c1], ps[:, c0:c1],
                            dinv[:, dcol * nblk + blk: dcol * nblk + blk + 1])
                    nc.sync.dma_start(
                        out=tbl_dst[nb0 - row_base:nb0 - row_base + P, 0:h2],
                        in_=tt[:, :])

            # ---------------------------------------------------------------
            def aggregate(tables, full_rows):
                """Gather + one-hot-matmul segment sums over the 4 edge
                streams (branch x src-half).  full_rows=False: branch b uses
                lhs cols/psum rows [b*h,(b+1)*h).  full_rows=True (layer 2):
                full 128-wide lhs, both branches accumulate into all rows."""
                nmask = [0]

                def build_mask(dstcol_ap):
                    mk = kp.tile([P, P], BF16, tag="mask")
                    mm = cfg.get("mask_mod", 3)
                    eng = nc.vector if (mm == 0 or nmask[0] % mm) \
                        else nc.gpsimd
                    nmask[0] += 1
                    eng.tensor_scalar(
                        out=mk[:], in0=iota_bf[:], scalar1=dstcol_ap,
                        scalar2=None, op0=mybir.AluOpType.is_equal)
                    return mk

                for hf in range(2):
                    for blocks in lay.groups:
                        bufs, dls = {}, {}
                        for b in range(2):
                            ch0 = int(lay.chunk_off[b, hf, blocks[0]])
                            ch1 = int(lay.chunk_off[b, hf, blocks[-1]]
                                      + lay.cap_chunks[b, hf, blocks[-1]])
                            nch = ch1 - ch0
                            it = idx_sb[b, hf][:, ch0 * 8: ch1 * 8]
                            dt = dst_sb[b, hf][:, ch0:ch1]
                            buf = mp.tile([P, nch, h2], BF16, tag=f"msg{b}")
                            nidx = nch * P
                            nc.gpsimd.dma_gather(
                                out_ap=buf[:], in_ap=tables[hf][:, :],
                                idxs_ap=it, num_idxs=nidx,
                                num_idxs_reg=nidx, elem_size=h2,
                                single_packet=(nidx <= 1024))
                            bufs[b] = (buf, ch0)
                            dls[b] = (dt, ch0)
                        for blk in blocks:
                            nb = slice(blk * P, (blk + 1) * P)
                            ps = psA.tile([P, P], F32, tag="agg")
                            for b in range(2):
                                buf, ch0 = bufs[b]
                                dt, dc0 = dls[b]
                                ncap = int(lay.cap_chunks[b, hf, blk])
                                co = int(lay.chunk_off[b, hf, blk])
                                for j in range(ncap):
                                    mk = build_mask(
                                        dt[:, co - dc0 + j: co - dc0 + j + 1])
                                    if full_rows:
                                        lh = buf[:, co - ch0 + j, 0:h2]
                                        o = ps[:, :]
                                        tpos = None
                                        st = (b == 0 and j == 0)
                                        sp = (b == 1 and j == ncap - 1)
                                    else:
                                        lh = buf[:, co - ch0 + j,
                                                 b * h:(b + 1) * h]
                                        o = ps[b * h:(b + 1) * h, :]
                                        tpos = (0, b * h)
                                        st = (j == 0)
                                        sp = (j == ncap - 1)
                                    nc.tensor.matmul(o, lhsT=lh, rhs=mk[:],
                                                     start=st, stop=sp,
                                                     tile_position=tpos)
                            if hf == 0:
                                nc.scalar.copy(aggT[:, nb], ps[:, :])
                            else:
                                nc.vector.tensor_add(out=aggT[:, nb],
                                                     in0=aggT[:, nb],
                                                     in1=ps[:, :])

            # ---------------------------------------------------------------
            def post01(layer, out_tile):
                nc.vector.tensor_tensor(out=xwT[:, :], in0=xwT[:, :],
                                        in1=dinvb[:, :],
                                        op=mybir.AluOpType.mult)
                nc.vector.tensor_tensor(out=aggT[:, :], in0=aggT[:, :],
                                        in1=xwT[:, :],
                                        op=mybir.AluOpType.add)
                nc.vector.tensor_tensor(out=aggT[:, :], in0=aggT[:, :],
                                        in1=dinvb[:, :],
                                        op=mybir.AluOpType.mult)
                nc.scalar.activation(out_tile[:, :], aggT[:, :],
                                     mybir.ActivationFunctionType.Relu,
                                     bias=biasv[:, layer:layer + 1])

            # ---------------------------------------------------------------
            def _phases():
                if nblk_pad > nloc:
                    nc.vector.memset(xwT[:, nloc:], 0.0)
                    nc.vector.memset(hT[:, nloc:], 0.0)
                    nc.vector.memset(h2T[:, nloc:], 0.0)

                # =================== layer 0 ===================
                with nc.named_scope("L0_tables"):
                    # zero padded tail rows of the half-1 local table once
                    zpad = hpad[1] * P - (nloc - hpad[0] * 1)
                    padrows = hpad[0] + hpad[1] - nloc      # 22
                    if padrows > 0:
                        zt = wp.tile([padrows, h2], BF16, tag="zpad")
                        nc.vector.memset(zt[:], 0.0)
                        nc.sync.dma_start(
                            out=tbl_loc_h[1][hpad[1] - padrows:hpad[1], :],
                            in_=zt[:])
                    NT = 512
                    for t0 in range(0, nloc, NT):
                        t1 = min(t0 + NT, nloc)
                        xts = []
                        for k in range(kch):
                            t = xp.tile([P, NT], BF16, tag=f"xb{k}")
                            nc.sync.dma_start(
                                out=t[:, :t1 - t0],
                                in_=xT_d[:, k * nloc + t0: k * nloc + t1])
                            xts.append(t)
                        # feature-major xw for the self-loop term
                        ps = psB.tile([P, NT], F32, tag="pst")
                        for k in range(kch):
                            nc.tensor.matmul(
                                ps[:h2, :t1 - t0],
                                lhsT=w0_sb[:, k * h2:(k + 1) * h2],
                                rhs=xts[k][:, :t1 - t0],
                                start=(k == 0), stop=(k == kch - 1))
                        nc.scalar.copy(xwT[:h2, t0:t1], ps[:h2, :t1 - t0])
                        # node-major prescaled table rows
                        for blk in range(t0 // P, cdiv(t1, P)):
                            nb0 = blk * P
                            nn = min(P, nloc - nb0)
                            psn = psN.tile([P, h2], F32, tag="psnm")
                            for k in range(kch):
                                nc.tensor.matmul(
                                    psn[:nn, :],
                                    lhsT=xts[k][:, nb0 - t0:nb0 - t0 + nn],
                                    rhs=w0_sb[:, k * h2:(k + 1) * h2],
                                    start=(k == 0), stop=(k == kch - 1))
                            tt = wp.tile([P, h2], BF16, tag="tblt")
                            for (cc0, cc1, dcol) in ((0, h, 0), (h, h2, 1)):
                                nc.vector.tensor_scalar_mul(
                                    tt[:nn, cc0:cc1], psn[:nn, cc0:cc1],
                                    dinv[:nn, dcol * nblk + blk:
                                         dcol * nblk + blk + 1])
                            hf = 0 if blk < hblk[0] else 1
                            r0 = nb0 - (hpad[0] if hf else 0)
                            nc.sync.dma_start(
                                out=tbl_loc_h[hf][r0:r0 + nn, :],
                                in_=tt[:nn, :])
                    if not no_coll:
                        for hf in range(2):
                            nc.gpsimd.collective_compute(
                                "AllGather", mybir.AluOpType.bypass,
                                replica_groups=[core_ids],
                                ins=[tbl_loc_h[hf][:]], outs=[tbl_half[hf][:]])
                with nc.named_scope("L0_agg"):
                    aggregate(tbl_half, False)
                with nc.named_scope("L0_post"):
                    post01(0, hT)
                if layers <= 1:
                    nc.sync.dma_start(out=out_d[0:P, :], in_=xwT[0:P, 0:c_out])
                    return

                # =================== layer 1 ===================
                with nc.named_scope("L1_tables"):
                    for hf in range(2):
                        build_table_rows(hT, w1_sb[:], tbl_loc_h[hf],
                                         0 if hf == 0 else hblk[0],
                                         hblk[0] if hf == 0 else nblk,
                                         0 if hf == 0 else hpad[0],
                                         ((0, h, 0), (h, h2, 1)))
                        if not no_coll:
                            nc.gpsimd.collective_compute(
                                "AllGather", mybir.AluOpType.bypass,
                                replica_groups=[core_ids],
                                ins=[tbl_loc_h[hf][:]], outs=[tbl_half[hf][:]])
                    build_xw_featmajor(lambda k, a, bb: hT[:, a:bb], 1,
                                       lambda k: w1_sb[:], h2, xwT)
                with nc.named_scope("L1_agg"):
                    aggregate(tbl_half, False)
                with nc.named_scope("L1_post"):
                    post01(1, h2T)
                if layers <= 2:
                    nc.sync.dma_start(out=out_d[0:P, :], in_=xwT[0:P, 0:c_out])
                    return

                # =================== layer 2 ===================
                with nc.named_scope("L2_tables"):
                    for hf in range(2):
                        build_table_rows(h2T, ident_bf[:], tbl_loc_h[hf],
                                         0 if hf == 0 else hblk[0],
                                         hblk[0] if hf == 0 else nblk,
                                         0 if hf == 0 else hpad[0],
                                         ((0, h2, 2),))
                        if not no_coll:
                            nc.gpsimd.collective_compute(
                                "AllGather", mybir.AluOpType.bypass,
                                replica_groups=[core_ids],
                                ins=[tbl_loc_h[hf][:]], outs=[tbl_half[hf][:]])
                    build_xw_featmajor(lambda k, a, bb: h2T[:, a:bb], 1,
                                       lambda k: wl_sb[:], c_out, xwT)
                with nc.named_scope("L2_agg"):
                    aggregate(tbl_half, True)

                # out16 = (WL^T aggT)*dinvall + xwTL*dinvall^2 + b_last
                with nc.named_scope("L2_post"):
                    NT = 512
                    for t0 in range(0, nblk_pad, NT):
                        t1 = min(t0 + NT, nblk_pad)
                        ps = psB.tile([P, NT], F32, tag="pst")
                        nc.tensor.matmul(ps[:c_out, :t1 - t0],
                                         lhsT=wlf_sb[:, :c_out],
                                         rhs=aggT[:, t0:t1],
                                         start=True, stop=True)
                        nc.scalar.copy(outTL[:, t0:t1], ps[:c_out, :t1 - t0])
                    nc.vector.tensor_tensor(out=xwTL[:, :], in0=xwTL[:, :],
                                            in1=dinvallb[:, :],
                                            op=mybir.AluOpType.mult)
                    nc.vector.tensor_tensor(out=outTL[:, :], in0=outTL[:, :],
                                            in1=xwTL[:, :],
                                            op=mybir.AluOpType.add)
                    nc.vector.tensor_tensor(out=outTL[:, :], in0=outTL[:, :],
                                            in1=dinvallb[:, :],
                                            op=mybir.AluOpType.mult)
                    nc.scalar.activation(outTL[:, :], outTL[:, :],
                                         mybir.ActivationFunctionType.Identity,
                                         bias=biasL[:, 0:1])

                with nc.named_scope("softmax"):
                    for blk in range(nblk):
                        nb0 = blk * P
                        nb1 = min(nb0 + P, nloc)
                        nn = nb1 - nb0
                        if nn <= 0:
                            continue
                        tp = psB.tile([P, c_out], F32, tag="pst")
                        nc.tensor.transpose(tp[:], outTL[:, nb0:nb0 + P],
                                            ident[:c_out, :c_out])
                        negmax = wp.tile([P, 1], F32, tag="negmax")
                        nc.vector.tensor_reduce(negmax[:], tp[:],
                                                axis=mybir.AxisListType.X,
                                                op=mybir.AluOpType.max,
                                                negate=True)
                        ex = wp.tile([P, c_out], F32, tag="ex")
                        nc.scalar.activation(ex[:], tp[:],
                                             mybir.ActivationFunctionType.Exp,
                                             bias=negmax[:, 0:1])
                        sume = wp.tile([P, 1], F32, tag="sume")
                        nc.vector.tensor_reduce(sume[:], ex[:],
                                                axis=mybir.AxisListType.X,
                                                op=mybir.AluOpType.add)
                        lse = wp.tile([P, 1], F32, tag="lse")
                        nc.scalar.activation(lse[:], sume[:],
                                             mybir.ActivationFunctionType.Ln)
                        fin = wp.tile([P, c_out], F32, tag="fin")
                        nc.vector.tensor_scalar(
                            out=fin[:], in0=tp[:], scalar1=negmax[:, 0:1],
                            scalar2=lse[:, 0:1], op0=mybir.AluOpType.add,
                            op1=mybir.AluOpType.subtract)
                        nc.sync.dma_start(out=out_d[nb0:nb1, :], in_=fin[:nn, :])

            for _rep in range(repeat):
                _phases()

    nc.compile()
    return nc


# ----------------------------------------------------------------------------
# driver
# ----------------------------------------------------------------------------

_CACHE = {}
_RUNNER = {}


def _build_runner(nc, n_cores):
    import jax
    from jax.sharding import Mesh, PartitionSpec
    from jax.experimental.shard_map import shard_map
    import concourse.mybir as mybir_
    from concourse import bass2jax
    from concourse.bass2jax import _bass_exec_p, partition_id_tensor

    bass2jax.install_neuronx_cc_hook()
    partition_name = (nc.partition_id_tensor.name
                      if nc.partition_id_tensor else None)
    in_names, out_names, out_avals, zero_outs = [], [], [], []
    for alloc in nc.m.functions[0].allocations:
        if not isinstance(alloc, mybir_.MemoryLocationSet):
            continue
        name = alloc.memorylocations[0].name
        if alloc.kind == "ExternalInput":
            if name != partition_name:
                in_names.append(name)
        elif alloc.kind == "ExternalOutput":
            out_names.append(name)
            shape = tuple(alloc.tensor_shape)
            dtype = mybir_.dt.np(alloc.dtype)
            out_avals.append(jax.core.ShapedArray(shape, dtype))
            zero_outs.append(np.zeros(shape, dtype))
    n_params = len(in_names)
    all_names = in_names + out_names
    if partition_name is not None:
        all_names.append(partition_name)

    def _body(*args):
        operands = list(args)
        if partition_name is not None:
            operands.append(partition_id_tensor())
        return tuple(_bass_exec_p.bind(
            *operands, out_avals=tuple(out_avals), in_names=tuple(all_names),
            out_names=tuple(out_names), lowering_input_output_aliases=(),
            sim_require_finite=True, sim_require_nnan=True, nc=nc))

    devices = jax.devices()[:n_cores]
    mesh = Mesh(np.asarray(devices), ("core",))
    n_out = len(out_names)
    fn = jax.jit(shard_map(_body, mesh=mesh,
                           in_specs=(PartitionSpec("core"),) * (n_params + n_out),
                           out_specs=(PartitionSpec("core"),) * n_out,
                           check_rep=False), keep_unused=True)
    return fn, in_names, out_names, out_avals, zero_outs, mesh


def _run_persistent(nc, in_maps, n_cores, key):
    import jax
    if key not in _RUNNER:
        fn, in_names, out_names, out_avals, zero_outs, mesh = \
            _build_runner(nc, n_cores)
        _RUNNER[key] = dict(fn=fn, in_names=in_names, out_names=out_names,
                            out_avals=out_avals, zero_outs=zero_outs,
                            mesh=mesh, dev_args=None)
    R = _RUNNER[key]
    concat_in = [np.concatenate([np.asarray(in_maps[c][nm])
                                 for c in range(n_cores)], axis=0)
                 for nm in R["in_names"]]
    concat_zero = [np.zeros((n_cores * z.shape[0], *z.shape[1:]), z.dtype)
                   for z in R["zero_outs"]]
    args = [jax.device_put(a) for a in concat_in + concat_zero]
    R["dev_args"] = args
    outs = R["fn"](*args)
    outs = [np.asarray(o) for o in outs]
    return {nm: outs[i].reshape(n_cores, *R["out_avals"][i].shape)
            for i, nm in enumerate(R["out_names"])}


def run(cfg, x, edge_index, is_reversed, weights, use_sim=False, repeat=1,
        layers=3, no_coll=False):
    lay, in_maps = host_prep(cfg, x, edge_index, is_reversed)
    wmap = host_prep_weights(cfg, **weights)
    for m in in_maps:
        m.update(wmap)

    sig = (lay.signature(), repeat, layers, no_coll)
    if sig in _CACHE:
        nc = _CACHE[sig]
    else:
        nc = build_program(cfg, lay, repeat=repeat, layers=layers,
                           no_coll=no_coll)
        _CACHE[sig] = nc

    n_cores = cfg["n_cores"]
    if use_sim:
        import concourse.bass_interp as bass_interp
        sim = bass_interp.MultiCoreSim(nc, n_cores, require_finite=False,
                                       require_nnan=False)
        for c in range(n_cores):
            for k, v in in_maps[c].items():
                sim.cores[c].tensor(k)[:] = v
        sim.simulate()
        outs = [np.array(sim.cores[c].tensor("out")) for c in range(n_cores)]
    else:
        key = str(sig)
        res = _run_persistent(nc, in_maps, n_cores, key)
        outs = list(res["out"])
    return np.concatenate(outs, axis=0)


def _marginal_sample(fn, args, iters):
    import jax, time as _t
    t0 = _t.time()
    o = fn(*args); jax.block_until_ready(o)
    base = _t.time() - t0
    t0 = _t.time()
    for _ in range(1 + iters):
        o = fn(*args)
    jax.block_until_ready(o)
    return (_t.time() - t0 - base) / iters * 1e9


def _marginal_ns(key, iters=6, reps=3):
    import jax
    R = _RUNNER[key]
    fn, args = R["fn"], R["dev_args"]
    o = fn(*args); jax.block_until_ready(o)
    return min(_marginal_sample(fn, args, iters) for _ in range(reps))


def time_device(inputs, iters=6, cfg=None, repeat_hi=4):
    """On-device execution time per kernel invocation: difference of
    in-program R-repeat vs 1-repeat marginal wall times cancels the
    fixed per-dispatch (axon RPC) overhead."""
    cfg = cfg or FULL_CFG
    weights = {k: np.asarray(inputs[k]) for k in
               ("W_st0", "b_st0", "W_ts0", "b_ts0", "W_st1", "b_st1",
                "W_ts1", "b_ts1", "W_last", "b_last")}
    keys = {}
    layers = int(__import__("os").environ.get("K2_LAYERS", "3"))
    no_coll = bool(int(__import__("os").environ.get("K2_NOCOLL", "0")))
    for rep in (1, repeat_hi):
        run(cfg, inputs["x"], inputs["edge_index"], inputs["is_reversed"],
            weights, repeat=rep, layers=layers, no_coll=no_coll)
        lay, _ = host_prep(cfg, inputs["x"], inputs["edge_index"],
                           inputs["is_reversed"])
        keys[rep] = str((lay.signature(), rep, layers, no_coll))
    import jax
    R1, RH = _RUNNER[keys[1]], _RUNNER[keys[repeat_hi]]
    for R in (R1, RH):
        o = R["fn"](*R["dev_args"]); jax.block_until_ready(o)
    m1s, mRs = [], []
    for _ in range(5):
        m1s.append(_marginal_sample(R1["fn"], R1["dev_args"], max(iters, 8)))
        mRs.append(_marginal_sample(RH["fn"], RH["dev_args"], max(iters, 8)))
    return (min(mRs) - min(m1s)) / (repeat_hi - 1)


def kernel(x, edge_index, is_reversed, W_st0, b_st0, W_ts0, b_ts0,
           W_st1, b_st1, W_ts1, b_ts1, W_last, b_last):
    cfg = FULL_CFG
    weights = dict(W_st0=W_st0, b_st0=b_st0, W_ts0=W_ts0, b_ts0=b_ts0,
                   W_st1=W_st1, b_st1=b_st1, W_ts1=W_ts1, b_ts1=b_ts1,
                   W_last=W_last, b_last=b_last)
    out = run(cfg, x, edge_index, is_reversed, weights)
    return out.astype(np.float32)
